# revision 8
# baseline (speedup 1.0000x reference)
"""Trainium2 Bass kernel for AspectNeighborAttention (gnn_message_passing).

Pure data-parallel over batch: 32 batches -> 8 NeuronCores x 4 batches.
All weights replicated, host-converted to bf16 and host-PRE-TRANSPOSED into
the chunk-major [128, KC, *] lhsT/rhs layouts the TensorEngine wants, so the
device does plain contiguous DMAs only. dep is host-bf16 (halves HBM traffic).

Per-core dataflow for each batch b (L=128 tokens, H=768, E=64, KC=6):
  zs^T   = Wz @ bertS^T + bz            (PE, bf16, packed PSUM groups)
  s_i,s_j= [wa_i;wa_j] @ zs^T           (PE, packed [1,128] regions)
  s_e    = reduce_e(dep * wa_e)         (DVE bf16 2x passes)
  score  = lrelu(s_i + s_j + s_e + ba)  (PE rank-1 bcast + DVE + ACT)
  attn   = mask * softmax(...)          (additive-shift masking, exp on ACT)
  D      = reduce_j(attn * dep)         (mult split Pool/DVE, bf16 2x reduce)
  nbr^T  = per-h-chunk matmuls from A/attn^T/D^T   (PE)
  temp   = nbr @ WhN^T + zs @ WhZ^T     (PE)
  out    = upd ? temp : bert            (blend, row-rolled DMA store)

The roll(z,-1)/roll(out,+1) pair is handled purely with shifted-row DMAs.
"""

import sys

for _p in ("/opt/trn_rl_repo",):
    if _p not in sys.path:
        sys.path.insert(0, _p)

import os
import numpy as np
import ml_dtypes

import concourse.bass as bass
import concourse.bacc as bacc_mod
import concourse.mybir as mybir
import concourse.tile as tile
from concourse.masks import make_identity

B, L, H, E = 32, 128, 768, 64
NCORES = 8
PB = B // NCORES  # batches per core
KC = H // 128     # 6 k-chunks
F32 = mybir.dt.float32
BF16 = mybir.dt.bfloat16
AF = mybir.ActivationFunctionType
OP = mybir.AluOpType
AX = mybir.AxisListType
MASK_SHIFT = 10000.0  # additive mask offset (see score masking)

_CACHED = {}

CFG = dict(
    dep_bufs=int(os.environ.get("K_DEP_BUFS", 3)),
    ttmp_bufs=int(os.environ.get("K_TTMP_BUFS", 3)),
    spool_bufs=int(os.environ.get("K_SPOOL_BUFS", 3)),
    opool_bufs=int(os.environ.get("K_OPOOL_BUFS", 2)),
    ptr_bufs=int(os.environ.get("K_PTR_BUFS", 2)),
    pbig_bufs=int(os.environ.get("K_PBIG_BUFS", 2)),
    jpd=int(os.environ.get("K_JPD", 56)),  # D-mult j-split: [0,jpd) Pool
    jp1=int(os.environ.get("K_JP1", 56)),  # s_e-mult j-split: [0,jp1) Pool
)


def _build(debug=False):
    nc = bacc_mod.Bacc("TRN2", target_bir_lowering=False, debug=False,
                       num_devices=NCORES)

    bert = nc.dram_tensor("bert", [PB, L, H], F32, kind="ExternalInput")
    bertsT = nc.dram_tensor("bertsT", [PB, 128, KC, 128], BF16,
                            kind="ExternalInput")
    dep = nc.dram_tensor("dep", [PB, L, L, E], BF16, kind="ExternalInput")
    adjf = nc.dram_tensor("adjf", [PB, L, L], F32, kind="ExternalInput")
    vrow = nc.dram_tensor("vrow", [1, PB, 128], F32, kind="ExternalInput")
    wzT_d = nc.dram_tensor("wzT", [128, KC, H], BF16, kind="ExternalInput")
    wfzT_d = nc.dram_tensor("wfzT", [128, KC, H], BF16, kind="ExternalInput")
    whnT_d = nc.dram_tensor("whnT", [128, KC, H], BF16, kind="ExternalInput")
    whzT_d = nc.dram_tensor("whzT", [128, KC, H], BF16, kind="ExternalInput")
    wfeT_d = nc.dram_tensor("wfeT", [E, H], BF16, kind="ExternalInput")
    w2T_d = nc.dram_tensor("w2T", [128, KC, 2], BF16, kind="ExternalInput")
    bzt = nc.dram_tensor("bzt", [1, H], BF16, kind="ExternalInput")
    wae = nc.dram_tensor("wae", [1, E], BF16, kind="ExternalInput")
    bat = nc.dram_tensor("bat", [1, 1], F32, kind="ExternalInput")
    out = nc.dram_tensor("out", [PB, L, H], F32, kind="ExternalOutput")

    dbg = {}
    if debug:
        for nm, shape, dt in [
            ("d_zsT", [128, KC, 128], BF16), ("d_si", [1, 128], F32),
            ("d_sjb", [1, 128], F32), ("d_se", [128, L], BF16),
            ("d_masked", [128, L], F32), ("d_attn", [128, L], BF16),
            ("d_dvec", [128, E], BF16), ("d_ab", [128, H], BF16),
            ("d_nbrT", [128, KC, 128], BF16), ("d_tempb", [128, H], F32),
            ("d_upd", [128, 1], F32), ("d_scb", [128, 128], F32),
        ]:
            dbg[nm] = nc.dram_tensor(nm, shape, dt, kind="ExternalOutput")
    with tile.TileContext(nc) as tc:
        with nc.allow_low_precision("bf16 softmax/D path, 2e-2 rel-err gate"):
            _body(tc, nc, bert, bertsT, dep, adjf, vrow, wzT_d, wfzT_d,
                  whnT_d, whzT_d, wfeT_d, w2T_d, bzt, wae, bat, out, dbg)
    nc.compile()
    return nc


def _body(tc, nc, bert, bertsT, dep, adjf, vrow, wzT_d, wfzT_d,
          whnT_d, whzT_d, wfeT_d, w2T_d, bzt, wae, bat, out, dbg=None):
    def dump(name, ap):
        if dbg and name in dbg:
            nc.sync.dma_start(dbg[name][...], ap)
    import contextlib
    cfg = CFG
    JPD = cfg["jpd"]
    JP1 = cfg["jp1"]
    ctx = contextlib.ExitStack()
    with ctx:
        wpool = ctx.enter_context(tc.tile_pool(name="weights", bufs=1))
        dpool = ctx.enter_context(
            tc.tile_pool(name="dep", bufs=cfg["dep_bufs"]))
        tpool = ctx.enter_context(
            tc.tile_pool(name="ttmp", bufs=cfg["ttmp_bufs"]))
        spool = ctx.enter_context(
            tc.tile_pool(name="small", bufs=cfg["spool_bufs"]))
        opool = ctx.enter_context(
            tc.tile_pool(name="outp", bufs=cfg["opool_bufs"]))
        p_tr = ctx.enter_context(
            tc.tile_pool(name="p_tr", bufs=cfg["ptr_bufs"], space="PSUM"))
        p_big = ctx.enter_context(
            tc.tile_pool(name="p_big", bufs=cfg["pbig_bufs"], space="PSUM"))

        # ---------------- one-time setup (plain DMAs only) ----------------
        wzT = wpool.tile([128, KC, H], BF16, tag="wzT")
        nc.sync.dma_start(wzT[:], wzT_d[...])
        wfzT = wpool.tile([128, KC, H], BF16, tag="wfzT")
        nc.sync.dma_start(wfzT[:], wfzT_d[...])
        whnT = wpool.tile([128, KC, H], BF16, tag="whnT")
        nc.sync.dma_start(whnT[:], whnT_d[...])
        whzT = wpool.tile([128, KC, H], BF16, tag="whzT")
        nc.sync.dma_start(whzT[:], whzT_d[...])
        wfeT = wpool.tile([E, H], BF16, tag="wfeT")
        nc.sync.dma_start(wfeT[:], wfeT_d[...])
        w2T = wpool.tile([128, KC, 2], BF16, tag="w2T")
        nc.sync.dma_start(w2T[:], w2T_d[...])
        bzr = wpool.tile([1, H], BF16, tag="bzr")
        nc.sync.dma_start(bzr[:], bzt[:, :])
        waer = wpool.tile([1, E], BF16, tag="waer")
        nc.sync.dma_start(waer[:], wae[:, :])
        bar = wpool.tile([1, 1], F32, tag="bar")
        nc.sync.dma_start(bar[:], bat[:, :])
        vrow4 = wpool.tile([1, PB, 128], F32, tag="vrow4")
        nc.sync.dma_start(vrow4[:], vrow[:, :, :])

        ones_f = wpool.tile([1, 128], F32, tag="ones_f")
        nc.gpsimd.memset(ones_f[:], 1.0)
        ones_b = wpool.tile([1, 128], BF16, tag="ones_b")
        nc.gpsimd.memset(ones_b[:], 1.0)
        id_bf = wpool.tile([128, 128], BF16, tag="id_bf")
        make_identity(nc, id_bf[:])

        # wa_e broadcast to all partitions via rank-1 matmul
        p_wae = p_tr.tile([128, 384], F32, tag="p_tr")
        nc.tensor.matmul(p_wae[:, 0:E], ones_b[:], waer[:],
                         start=True, stop=True)
        wae_bc = wpool.tile([128, E], BF16, tag="wae_bc")
        nc.scalar.copy(wae_bc[:], p_wae[:, 0:E])

        # ---------------- per-batch pipeline ----------------
        for b in range(PB):
            # bertS: rows shifted by one token (z roll); f32 exact for blend
            bertS = spool.tile([128, H], F32, tag="bertS")
            nc.scalar.dma_start(bertS[0:127, :], bert[b, 1:128, :])
            nc.scalar.dma_start(bertS[127:128, :], bert[b, 0:1, :])
            bertST = spool.tile([128, KC, 128], BF16, tag="bertST")
            nc.scalar.dma_start(bertST[:], bertsT[b, :, :, :])

            dept = dpool.tile([128, L, E], BF16, tag="dept")
            nc.sync.dma_start(dept[:, 0:64, :], dep[b, :, 0:64, :])
            nc.sync.dma_start(dept[:, 64:L, :], dep[b, :, 64:L, :])
            adjt = spool.tile([128, L], F32, tag="adjt")
            nc.scalar.dma_start(adjt[:], adjf[b, :, :])

            # ---- zs^T = Wz @ bertS^T + bz: 6 groups packed in one PSUM ----
            p_z = p_big.tile([128, H], F32, tag="p_big")
            for hc in range(KC):
                ns = slice(hc * 128, (hc + 1) * 128)
                for kc in range(KC):
                    nc.tensor.matmul(p_z[:, ns], wzT[:, kc, ns],
                                     bertST[:, kc, :],
                                     start=(kc == 0), stop=False)
                nc.tensor.matmul(p_z[:, ns], bzr[0:1, ns], ones_b[:],
                                 start=False, stop=True)
            zsT = spool.tile([128, KC, 128], BF16, tag="zsT")
            nc.scalar.copy(zsT[:], p_z[:])
            if b == 0:
                dump("d_zsT", zsT[:])

            # ---- s_i, s_j, score-base packed into one p_tr tile ----
            p_s3 = p_tr.tile([128, 384], F32, tag="p_tr")
            for kc in range(KC):
                nc.tensor.matmul(p_s3[0:1, 0:128], w2T[:, kc, 0:1],
                                 zsT[:, kc, :],
                                 start=(kc == 0), stop=(kc == KC - 1))
            for kc in range(KC):
                nc.tensor.matmul(p_s3[0:1, 128:256], w2T[:, kc, 1:2],
                                 zsT[:, kc, :],
                                 start=(kc == 0), stop=(kc == KC - 1))
            si_row = spool.tile([1, 128], F32, tag="si_row")
            nc.scalar.copy(si_row[:], p_s3[0:1, 0:128])
            sjb = spool.tile([1, 128], F32, tag="sjb")
            nc.vector.tensor_scalar(sjb[:], p_s3[0:1, 128:256], bar[0:1, 0:1],
                                    None, op0=OP.add)
            # score base: s_i (row-bcast) + (s_j + ba) (col-bcast)
            nc.tensor.matmul(p_s3[:, 256:384], si_row[:], ones_f[:],
                             start=True, stop=False)
            nc.tensor.matmul(p_s3[:, 256:384], ones_f[:], sjb[:],
                             start=False, stop=True)
            if b == 0:
                dump("d_si", si_row[:])
                dump("d_sjb", sjb[:])

            # ---- s_e = reduce_e(dep * wa_e); mult split Pool/DVE ----
            tmp1 = tpool.tile([128, L, E], BF16, tag="ttmp")
            nc.gpsimd.tensor_tensor(
                tmp1[:, 0:JP1, :], dept[:, 0:JP1, :],
                wae_bc[:].unsqueeze(1).broadcast_to([128, JP1, E]),
                op=OP.mult)
            nc.vector.tensor_tensor(
                tmp1[:, JP1:L, :], dept[:, JP1:L, :],
                wae_bc[:].unsqueeze(1).broadcast_to([128, L - JP1, E]),
                op=OP.mult)
            se = spool.tile([128, L], BF16, tag="se")
            nc.vector.tensor_reduce(se[:], tmp1[:], axis=AX.X, op=OP.add)
            if b == 0:
                dump("d_se", se[:])
                scb_s = spool.tile([128, 128], F32, tag="scb_s")
                nc.vector.tensor_copy(scb_s[:], p_s3[:, 256:384])
                dump("d_scb", scb_s[:])

            # ---- score = lrelu(se + base); masked = (score+C)*m ----
            sadd = spool.tile([128, L], F32, tag="sadd")
            nc.vector.tensor_tensor(sadd[:], se[:], p_s3[:, 256:384],
                                    op=OP.add)
            score = spool.tile([128, L], F32, tag="score")
            nc.scalar.activation(score[:], sadd[:], AF.Lrelu, alpha=0.01)
            masked = spool.tile([128, L], F32, tag="masked")
            nc.vector.scalar_tensor_tensor(
                masked[:], score[:], MASK_SHIFT, adjt[:],
                op0=OP.add, op1=OP.mult)
            if b == 0:
                dump("d_masked", masked[:])

            # ---- softmax over j (free axis); attn emitted directly bf16 ----
            mxn = spool.tile([128, 1], F32, tag="mxn")
            nc.vector.tensor_reduce(mxn[:], masked[:], axis=AX.X, op=OP.max,
                                    negate=True)
            ex = spool.tile([128, L], F32, tag="ex")
            sumex = spool.tile([128, 1], F32, tag="sumex")
            nc.scalar.activation(ex[:], masked[:], AF.Exp, bias=mxn[:],
                                 scale=1.0, accum_out=sumex[:])
            rec = spool.tile([128, 1], F32, tag="rec")
            nc.vector.reciprocal(rec[:], sumex[:])
            attnb = spool.tile([128, L], BF16, tag="attnb")
            nc.vector.scalar_tensor_tensor(
                attnb[:], ex[:], rec[:], adjt[:], op0=OP.mult, op1=OP.mult)
            if b == 0:
                dump("d_attn", attnb[:])

            # ---- D = reduce_j(attn * dep) in natural [i,j,e] layout ----
            # mult: contiguous, attn broadcast along inner e; split Pool/DVE
            tmp2 = tpool.tile([128, L, E], BF16, tag="ttmp")
            nc.gpsimd.tensor_tensor(
                tmp2[:, 0:JPD, :], dept[:, 0:JPD, :],
                attnb[:, 0:JPD].unsqueeze(2).broadcast_to([128, JPD, E]),
                op=OP.mult)
            nc.vector.tensor_tensor(
                tmp2[:, JPD:L, :], dept[:, JPD:L, :],
                attnb[:, JPD:L].unsqueeze(2).broadcast_to([128, L - JPD, E]),
                op=OP.mult)
            # pairwise tree over j: Pool reduces j[0:64), DVE j[64:128)
            h = 32
            while h >= 1:
                nc.gpsimd.tensor_tensor(
                    tmp2[:, 0:h, :], tmp2[:, 0:h, :], tmp2[:, h:2 * h, :],
                    op=OP.add)
                nc.vector.tensor_tensor(
                    tmp2[:, 64:64 + h, :], tmp2[:, 64:64 + h, :],
                    tmp2[:, 64 + h:64 + 2 * h, :], op=OP.add)
                h //= 2
            dvb = spool.tile([128, E], BF16, tag="dvb")
            nc.vector.tensor_tensor(dvb[:], tmp2[:, 0, :], tmp2[:, 64, :],
                                    op=OP.add)
            if b == 0:
                dump("d_dvec", dvb[:])

            # attn^T and D^T via PE transposes into one packed PSUM tile
            p_ad = p_tr.tile([128, 256], BF16, tag="p_tr")
            nc.tensor.transpose(p_ad[:, 0:128], attnb[:], id_bf[:])
            nc.tensor.transpose(p_ad[0:E, 128:256], dvb[:], id_bf[:])
            attnT = spool.tile([128, 128], BF16, tag="attnT")
            nc.scalar.copy(attnT[:], p_ad[:, 0:128])
            dT = spool.tile([E, 128], BF16, tag="dT")
            nc.scalar.copy(dT[:], p_ad[0:E, 128:256])

            # ---- A = zs @ WfZ^T  ([j, h], bf16) ----
            p_a = p_big.tile([128, H], F32, tag="p_big")
            for ns in (slice(0, 512), slice(512, H)):
                for kc in range(KC):
                    nc.tensor.matmul(p_a[:, ns], zsT[:, kc, :],
                                     wfzT[:, kc, ns],
                                     start=(kc == 0), stop=(kc == KC - 1))
            ab = spool.tile([128, H], BF16, tag="ab")
            nc.scalar.copy(ab[:], p_a[:])
            if b == 0:
                dump("d_ab", ab[:])

            # ---- nbr^T per h-chunk packed into one PSUM tile ----
            p_n = p_big.tile([128, H], F32, tag="p_big")
            for hc in range(KC):
                ns = slice(hc * 128, (hc + 1) * 128)
                nc.tensor.matmul(p_n[:, ns], ab[:, ns], attnT[:],
                                 start=True, stop=False)
                nc.tensor.matmul(p_n[:, ns], wfeT[:, ns], dT[:],
                                 start=False, stop=True)
            nbrT = spool.tile([128, KC, 128], BF16, tag="nbrT")
            nc.scalar.copy(nbrT[:], p_n[:])
            if b == 0:
                dump("d_nbrT", nbrT[:])

            # ---- temp = nbr @ WhN^T + zs @ WhZ^T ----
            p_t = p_big.tile([128, H], F32, tag="p_big")
            for ns in (slice(0, 512), slice(512, H)):
                for kc in range(KC):
                    nc.tensor.matmul(p_t[:, ns], nbrT[:, kc, :],
                                     whnT[:, kc, ns],
                                     start=(kc == 0), stop=False)
                for kc in range(KC):
                    nc.tensor.matmul(p_t[:, ns], zsT[:, kc, :],
                                     whzT[:, kc, ns],
                                     start=False, stop=(kc == KC - 1))
            tempb = opool.tile([128, H], F32, tag="tempb")
            nc.scalar.copy(tempb[:], p_t[:])
            if b == 0:
                dump("d_tempb", tempb[:])

            # ---- upd mask: span-row (host) -> column via rank-1; & any_j ----
            p_v = p_tr.tile([128, 384], F32, tag="p_tr")
            nc.tensor.matmul(p_v[:, 0:1], vrow4[0:1, b, :], ones_f[0:1, 0:1],
                             start=True, stop=True)
            anynb = spool.tile([128, 1], F32, tag="anynb")
            nc.vector.tensor_reduce(anynb[:], adjt[:], axis=AX.X, op=OP.max)
            upd = spool.tile([128, 1], F32, tag="upd")
            nc.vector.tensor_tensor(upd[:], p_v[:, 0:1], anynb[:], op=OP.mult)
            if b == 0:
                dump("d_upd", upd[:])

            # ---- blend + rolled store ----
            tdiff = opool.tile([128, H], F32, tag="tdiff")
            nc.gpsimd.tensor_tensor(tdiff[:], tempb[:], bertS[:],
                                    op=OP.subtract)
            outt = opool.tile([128, H], F32, tag="outt")
            nc.vector.scalar_tensor_tensor(
                outt[:], tdiff[:], upd[:], bertS[:], op0=OP.mult, op1=OP.add)
            nc.scalar.dma_start(out[b, 1:128, :], outt[0:127, :])
            nc.scalar.dma_start(out[b, 0:1, :], outt[127:128, :])


def _get_nc():
    if "nc" not in _CACHED:
        _CACHED["nc"] = _build()
    return _CACHED["nc"]


def _chunkT(w):
    """W [rows, K] -> W^T chunk-major [128, K//128, rows] (lhsT layout)."""
    rows, k = w.shape
    return np.ascontiguousarray(
        w.T.reshape(k // 128, 128, rows).transpose(1, 0, 2))


def _prep_in_maps(bert_hidden_states, dep_type_adj, deprel_adj,
                  asp_start, asp_end, Wz, bz, wa, ba, Wf, Wh):
    bf = ml_dtypes.bfloat16
    bert = np.ascontiguousarray(np.asarray(bert_hidden_states, np.float32))
    dep = np.asarray(dep_type_adj, np.float32).astype(bf)
    adjf = np.ascontiguousarray(np.asarray(deprel_adj).astype(np.float32))
    # bertS^T chunk-major per batch: rows shifted by one (the z-roll)
    bs = np.roll(bert, -1, axis=1)
    bertsT = np.ascontiguousarray(
        bs.transpose(0, 2, 1).reshape(B, KC, 128, L).transpose(0, 2, 1, 3)
    ).astype(bf)
    pos = np.arange(L, dtype=np.float32)
    s_ = np.asarray(asp_start).astype(np.float32)[:, None]
    e_ = np.asarray(asp_end).astype(np.float32)[:, None]
    vrow_full = ((pos[None, :] >= s_) & (pos[None, :] <= e_)).astype(np.float32)

    Wz = np.asarray(Wz, np.float32)
    Wf = np.asarray(Wf, np.float32)
    Wh = np.asarray(Wh, np.float32)
    wa_f = np.asarray(wa, np.float32)
    wzT = _chunkT(Wz).astype(bf)
    wfzT = _chunkT(Wf[:, :H]).astype(bf)
    whnT = _chunkT(Wh[:, :H]).astype(bf)
    whzT = _chunkT(Wh[:, H:]).astype(bf)
    wfeT = np.ascontiguousarray(Wf[:, H:].T).astype(bf)
    w2T = _chunkT(wa_f[:2 * H].reshape(2, H)).astype(bf)
    bzb = np.asarray(bz, np.float32)[None, :].astype(bf)
    waeb = wa_f[2 * H:][None, :].astype(bf)
    bab = np.asarray(ba, np.float32).reshape(1, 1)

    in_maps = []
    for c in range(NCORES):
        s = slice(c * PB, (c + 1) * PB)
        in_maps.append(dict(
            bert=bert[s], bertsT=np.ascontiguousarray(bertsT[s]),
            dep=dep[s], adjf=adjf[s],
            vrow=np.ascontiguousarray(vrow_full[s][None, :, :]),
            wzT=wzT, wfzT=wfzT, whnT=whnT, whzT=whzT, wfeT=wfeT, w2T=w2T,
            bzt=bzb, wae=waeb, bat=bab,
        ))
    return in_maps


def kernel(bert_hidden_states, dep_type_adj, deprel_adj, asp_start, asp_end,
           Wz, bz, wa, ba, Wf, Wh):
    from concourse.bass_utils import run_bass_kernel_spmd

    in_maps = _prep_in_maps(bert_hidden_states, dep_type_adj, deprel_adj,
                            asp_start, asp_end, Wz, bz, wa, ba, Wf, Wh)
    nc = _get_nc()
    res = run_bass_kernel_spmd(nc, in_maps, core_ids=list(range(NCORES)),
                               trace=bool(_CACHED.get("trace")),
                               tmpdir=_CACHED.get("trace_tmpdir"))
    _CACHED["last_results"] = res
    outs = [res.results[c]["out"] for c in range(NCORES)]
    return np.concatenate(outs, axis=0).astype(np.float32)



# revision 11
# speedup vs baseline: 1.1182x; 1.1182x over previous
"""Trainium2 Bass kernel for AspectNeighborAttention (gnn_message_passing).

Pure data-parallel over batch: 32 batches -> 8 NeuronCores x 4 batches.
All weights replicated, host-converted to bf16 and host-PRE-TRANSPOSED into
the chunk-major [128, KC, *] lhsT/rhs layouts the TensorEngine wants, so the
device does plain contiguous DMAs only. dep is host-bf16 (halves HBM traffic).

Per-core dataflow for each batch b (L=128 tokens, H=768, E=64, KC=6):
  zs^T   = Wz @ bertS^T + bz            (PE, bf16, packed PSUM groups)
  s_i,s_j= [wa_i;wa_j] @ zs^T           (PE, packed [1,128] regions)
  s_e    = reduce_e(dep * wa_e)         (DVE bf16 2x passes)
  score  = lrelu(s_i + s_j + s_e + ba)  (PE rank-1 bcast + DVE + ACT)
  attn   = mask * softmax(...)          (additive-shift masking, exp on ACT)
  D      = reduce_j(attn * dep)         (mult split Pool/DVE, bf16 2x reduce)
  nbr^T  = per-h-chunk matmuls from A/attn^T/D^T   (PE)
  temp   = nbr @ WhN^T + zs @ WhZ^T     (PE)
  out    = upd ? temp : bert            (blend, row-rolled DMA store)

The roll(z,-1)/roll(out,+1) pair is handled purely with shifted-row DMAs.
"""

import sys

for _p in ("/opt/trn_rl_repo",):
    if _p not in sys.path:
        sys.path.insert(0, _p)

import os
import numpy as np
import ml_dtypes

import concourse.bass as bass
import concourse.bacc as bacc_mod
import concourse.mybir as mybir
import concourse.tile as tile
from concourse.masks import make_identity

B, L, H, E = 32, 128, 768, 64
NCORES = 8
PB = B // NCORES  # batches per core
KC = H // 128     # 6 k-chunks
F32 = mybir.dt.float32
BF16 = mybir.dt.bfloat16
AF = mybir.ActivationFunctionType
OP = mybir.AluOpType
AX = mybir.AxisListType
MASK_SHIFT = 10000.0  # additive mask offset (see score masking)

_CACHED = {}

CFG = dict(
    dep_bufs=int(os.environ.get("K_DEP_BUFS", 3)),
    ttmp_bufs=int(os.environ.get("K_TTMP_BUFS", 3)),
    spool_bufs=int(os.environ.get("K_SPOOL_BUFS", 3)),
    opool_bufs=int(os.environ.get("K_OPOOL_BUFS", 2)),
    ptr_bufs=int(os.environ.get("K_PTR_BUFS", 2)),
    pbig_bufs=int(os.environ.get("K_PBIG_BUFS", 2)),
    jpd=int(os.environ.get("K_JPD", 56)),  # D-mult j-split: [0,jpd) Pool
    jp1=int(os.environ.get("K_JP1", 56)),  # s_e-mult j-split: [0,jp1) Pool
)


def _build(debug=False):
    nc = bacc_mod.Bacc("TRN2", target_bir_lowering=False, debug=False,
                       num_devices=NCORES)

    bert = nc.dram_tensor("bert", [PB, L, H], F32, kind="ExternalInput")
    bertsT = nc.dram_tensor("bertsT", [PB, 128, KC, 128], BF16,
                            kind="ExternalInput")
    dep = nc.dram_tensor("dep", [PB, L, L, E], BF16, kind="ExternalInput")
    adjf = nc.dram_tensor("adjf", [PB, L, L], F32, kind="ExternalInput")
    vrow = nc.dram_tensor("vrow", [1, PB, 128], F32, kind="ExternalInput")
    wzT_d = nc.dram_tensor("wzT", [128, KC, H], BF16, kind="ExternalInput")
    wfzT_d = nc.dram_tensor("wfzT", [128, KC, H], BF16, kind="ExternalInput")
    whnT_d = nc.dram_tensor("whnT", [128, KC, H], BF16, kind="ExternalInput")
    whzT_d = nc.dram_tensor("whzT", [128, KC, H], BF16, kind="ExternalInput")
    wfeT_d = nc.dram_tensor("wfeT", [E, H], BF16, kind="ExternalInput")
    w2T_d = nc.dram_tensor("w2T", [128, KC, 2], BF16, kind="ExternalInput")
    bzt = nc.dram_tensor("bzt", [1, H], BF16, kind="ExternalInput")
    wae = nc.dram_tensor("wae", [1, E], BF16, kind="ExternalInput")
    bat = nc.dram_tensor("bat", [1, 1], F32, kind="ExternalInput")
    out = nc.dram_tensor("out", [PB, L, H], F32, kind="ExternalOutput")

    dbg = {}
    if debug:
        for nm, shape, dt in [
            ("d_zsT", [128, KC, 128], BF16), ("d_si", [1, 128], F32),
            ("d_sjb", [1, 128], F32), ("d_se", [128, L], BF16),
            ("d_masked", [128, L], F32), ("d_attn", [128, L], BF16),
            ("d_dvec", [128, E], BF16), ("d_ab", [128, H], BF16),
            ("d_nbrT", [128, KC, 128], BF16), ("d_tempb", [128, H], F32),
            ("d_upd", [128, 1], F32), ("d_scb", [128, 128], F32),
        ]:
            dbg[nm] = nc.dram_tensor(nm, shape, dt, kind="ExternalOutput")
    with tile.TileContext(nc) as tc:
        with nc.allow_low_precision("bf16 softmax/D path, 2e-2 rel-err gate"):
            _body(tc, nc, bert, bertsT, dep, adjf, vrow, wzT_d, wfzT_d,
                  whnT_d, whzT_d, wfeT_d, w2T_d, bzt, wae, bat, out, dbg)
    nc.compile()
    return nc


def _body(tc, nc, bert, bertsT, dep, adjf, vrow, wzT_d, wfzT_d,
          whnT_d, whzT_d, wfeT_d, w2T_d, bzt, wae, bat, out, dbg=None):
    def dump(name, ap):
        if dbg and name in dbg:
            nc.sync.dma_start(dbg[name][...], ap)
    import contextlib
    cfg = CFG
    JPD = cfg["jpd"]
    JP1 = cfg["jp1"]
    ctx = contextlib.ExitStack()
    with ctx:
        wpool = ctx.enter_context(tc.tile_pool(name="weights", bufs=1))
        dpool = ctx.enter_context(
            tc.tile_pool(name="dep", bufs=cfg["dep_bufs"]))
        tpool = ctx.enter_context(
            tc.tile_pool(name="ttmp", bufs=cfg["ttmp_bufs"]))
        spool = ctx.enter_context(
            tc.tile_pool(name="small", bufs=cfg["spool_bufs"]))
        opool = ctx.enter_context(
            tc.tile_pool(name="outp", bufs=cfg["opool_bufs"]))
        p_tr = ctx.enter_context(
            tc.tile_pool(name="p_tr", bufs=cfg["ptr_bufs"], space="PSUM"))
        p_big = ctx.enter_context(
            tc.tile_pool(name="p_big", bufs=cfg["pbig_bufs"], space="PSUM"))

        # ---------------- one-time setup (plain DMAs only) ----------------
        wzT = wpool.tile([128, KC, H], BF16, tag="wzT")
        nc.sync.dma_start(wzT[:], wzT_d[...])
        wfzT = wpool.tile([128, KC, H], BF16, tag="wfzT")
        nc.sync.dma_start(wfzT[:], wfzT_d[...])
        whnT = wpool.tile([128, KC, H], BF16, tag="whnT")
        nc.sync.dma_start(whnT[:], whnT_d[...])
        whzT = wpool.tile([128, KC, H], BF16, tag="whzT")
        nc.sync.dma_start(whzT[:], whzT_d[...])
        wfeT = wpool.tile([E, H], BF16, tag="wfeT")
        nc.sync.dma_start(wfeT[:], wfeT_d[...])
        w2T = wpool.tile([128, KC, 2], BF16, tag="w2T")
        nc.sync.dma_start(w2T[:], w2T_d[...])
        bzr = wpool.tile([1, H], BF16, tag="bzr")
        nc.sync.dma_start(bzr[:], bzt[:, :])
        waer = wpool.tile([1, E], BF16, tag="waer")
        nc.sync.dma_start(waer[:], wae[:, :])
        bar = wpool.tile([1, 1], F32, tag="bar")
        nc.sync.dma_start(bar[:], bat[:, :])
        vrow4 = wpool.tile([1, PB, 128], F32, tag="vrow4")
        nc.sync.dma_start(vrow4[:], vrow[:, :, :])

        ones_f = wpool.tile([1, 128], F32, tag="ones_f")
        nc.gpsimd.memset(ones_f[:], 1.0)
        ones_b = wpool.tile([1, 128], BF16, tag="ones_b")
        nc.gpsimd.memset(ones_b[:], 1.0)
        id_bf = wpool.tile([128, 128], BF16, tag="id_bf")
        make_identity(nc, id_bf[:])

        # wa_e broadcast to all partitions via rank-1 matmul
        p_wae = p_tr.tile([128, 384], F32, tag="p_tr")
        nc.tensor.matmul(p_wae[:, 0:E], ones_b[:], waer[:],
                         start=True, stop=True)
        wae_bc = wpool.tile([128, E], BF16, tag="wae_bc")
        nc.scalar.copy(wae_bc[:], p_wae[:, 0:E])

        # ---------------- per-batch pipeline ----------------
        for b in range(PB):
            # bertS: rows shifted by one token (z roll); f32 exact for blend
            bertS = spool.tile([128, H], F32, tag="bertS")
            nc.sync.dma_start(bertS[0:127, :], bert[b, 1:128, :])
            nc.sync.dma_start(bertS[127:128, :], bert[b, 0:1, :])
            bertST = spool.tile([128, KC, 128], BF16, tag="bertST")
            nc.sync.dma_start(bertST[:], bertsT[b, :, :, :])

            dept = dpool.tile([128, L, E], BF16, tag="dept")
            nc.sync.dma_start(dept[:, 0:64, :], dep[b, :, 0:64, :])
            nc.sync.dma_start(dept[:, 64:L, :], dep[b, :, 64:L, :])
            adjt = spool.tile([128, L], F32, tag="adjt")
            nc.sync.dma_start(adjt[:], adjf[b, :, :])

            # ---- zs^T = Wz @ bertS^T + bz: 6 groups packed in one PSUM ----
            p_z = p_big.tile([128, H], F32, tag="p_big")
            for hc in range(KC):
                ns = slice(hc * 128, (hc + 1) * 128)
                for kc in range(KC):
                    nc.tensor.matmul(p_z[:, ns], wzT[:, kc, ns],
                                     bertST[:, kc, :],
                                     start=(kc == 0), stop=False)
                nc.tensor.matmul(p_z[:, ns], bzr[0:1, ns], ones_b[:],
                                 start=False, stop=True)
            zsT = spool.tile([128, KC, 128], BF16, tag="zsT")
            nc.scalar.copy(zsT[:], p_z[:])
            if b == 0:
                dump("d_zsT", zsT[:])

            # ---- s_i, s_j, score-base packed into one p_tr tile ----
            p_s3 = p_tr.tile([128, 384], F32, tag="p_tr")
            for kc in range(KC):
                nc.tensor.matmul(p_s3[0:1, 0:128], w2T[:, kc, 0:1],
                                 zsT[:, kc, :],
                                 start=(kc == 0), stop=(kc == KC - 1))
            for kc in range(KC):
                nc.tensor.matmul(p_s3[0:1, 128:256], w2T[:, kc, 1:2],
                                 zsT[:, kc, :],
                                 start=(kc == 0), stop=(kc == KC - 1))
            si_row = spool.tile([1, 128], F32, tag="si_row")
            nc.scalar.copy(si_row[:], p_s3[0:1, 0:128])
            sjb = spool.tile([1, 128], F32, tag="sjb")
            nc.vector.tensor_scalar(sjb[:], p_s3[0:1, 128:256], bar[0:1, 0:1],
                                    None, op0=OP.add)
            # score base: s_i (row-bcast) + (s_j + ba) (col-bcast)
            nc.tensor.matmul(p_s3[:, 256:384], si_row[:], ones_f[:],
                             start=True, stop=False)
            nc.tensor.matmul(p_s3[:, 256:384], ones_f[:], sjb[:],
                             start=False, stop=True)
            if b == 0:
                dump("d_si", si_row[:])
                dump("d_sjb", sjb[:])

            # ---- s_e = reduce_e(dep * wa_e); mult split Pool/DVE ----
            tmp1 = tpool.tile([128, L, E], BF16, tag="ttmp")
            nc.gpsimd.tensor_tensor(
                tmp1[:, 0:JP1, :], dept[:, 0:JP1, :],
                wae_bc[:].unsqueeze(1).broadcast_to([128, JP1, E]),
                op=OP.mult)
            nc.vector.tensor_tensor(
                tmp1[:, JP1:L, :], dept[:, JP1:L, :],
                wae_bc[:].unsqueeze(1).broadcast_to([128, L - JP1, E]),
                op=OP.mult)
            se = spool.tile([128, L], BF16, tag="se")
            nc.vector.tensor_reduce(se[:], tmp1[:], axis=AX.X, op=OP.add)
            if b == 0:
                dump("d_se", se[:])
                scb_s = spool.tile([128, 128], F32, tag="scb_s")
                nc.vector.tensor_copy(scb_s[:], p_s3[:, 256:384])
                dump("d_scb", scb_s[:])

            # ---- score = lrelu(se + base); masked = (score+C)*m ----
            sadd = spool.tile([128, L], F32, tag="sadd")
            nc.vector.tensor_tensor(sadd[:], se[:], p_s3[:, 256:384],
                                    op=OP.add)
            score = spool.tile([128, L], F32, tag="score")
            nc.scalar.activation(score[:], sadd[:], AF.Lrelu, alpha=0.01)
            masked = spool.tile([128, L], F32, tag="masked")
            nc.vector.scalar_tensor_tensor(
                masked[:], score[:], MASK_SHIFT, adjt[:],
                op0=OP.add, op1=OP.mult)
            if b == 0:
                dump("d_masked", masked[:])

            # ---- softmax over j (free axis); attn emitted directly bf16 ----
            mxn = spool.tile([128, 1], F32, tag="mxn")
            nc.vector.tensor_reduce(mxn[:], masked[:], axis=AX.X, op=OP.max,
                                    negate=True)
            ex = spool.tile([128, L], F32, tag="ex")
            sumex = spool.tile([128, 1], F32, tag="sumex")
            nc.scalar.activation(ex[:], masked[:], AF.Exp, bias=mxn[:],
                                 scale=1.0, accum_out=sumex[:])
            rec = spool.tile([128, 1], F32, tag="rec")
            nc.vector.reciprocal(rec[:], sumex[:])
            attnb = spool.tile([128, L], BF16, tag="attnb")
            nc.vector.scalar_tensor_tensor(
                attnb[:], ex[:], rec[:], adjt[:], op0=OP.mult, op1=OP.mult)
            if b == 0:
                dump("d_attn", attnb[:])

            # ---- D = reduce_j(attn * dep) in natural [i,j,e] layout ----
            # mult: contiguous, attn broadcast along inner e; split Pool/DVE
            tmp2 = tpool.tile([128, L, E], BF16, tag="ttmp")
            nc.gpsimd.tensor_tensor(
                tmp2[:, 0:JPD, :], dept[:, 0:JPD, :],
                attnb[:, 0:JPD].unsqueeze(2).broadcast_to([128, JPD, E]),
                op=OP.mult)
            nc.vector.tensor_tensor(
                tmp2[:, JPD:L, :], dept[:, JPD:L, :],
                attnb[:, JPD:L].unsqueeze(2).broadcast_to([128, L - JPD, E]),
                op=OP.mult)
            # pairwise tree over j, ping-ponged through tmp3 (no aliasing):
            # Pool reduces j[0:64) -> row 0, DVE j[64:128) -> row 64.
            tmp3 = tpool.tile([128, L, E], BF16, tag="ttmp")
            src, dst = tmp2, tmp3
            h = 32
            while h >= 1:
                nc.gpsimd.tensor_tensor(
                    dst[:, 0:h, :], src[:, 0:h, :], src[:, h:2 * h, :],
                    op=OP.add)
                nc.vector.tensor_tensor(
                    dst[:, 64:64 + h, :], src[:, 64:64 + h, :],
                    src[:, 64 + h:64 + 2 * h, :], op=OP.add)
                src, dst = dst, src
                h //= 2
            dvb = spool.tile([128, E], BF16, tag="dvb")
            nc.vector.tensor_tensor(dvb[:], src[:, 0, :], src[:, 64, :],
                                    op=OP.add)
            if b == 0:
                dump("d_dvec", dvb[:])

            # attn^T and D^T via PE transposes into one packed PSUM tile
            p_ad = p_tr.tile([128, 256], BF16, tag="p_tr")
            nc.tensor.transpose(p_ad[:, 0:128], attnb[:], id_bf[:])
            nc.tensor.transpose(p_ad[0:E, 128:256], dvb[:], id_bf[:])
            attnT = spool.tile([128, 128], BF16, tag="attnT")
            nc.scalar.copy(attnT[:], p_ad[:, 0:128])
            dT = spool.tile([E, 128], BF16, tag="dT")
            nc.scalar.copy(dT[:], p_ad[0:E, 128:256])

            # ---- A = zs @ WfZ^T  ([j, h], bf16) ----
            p_a = p_big.tile([128, H], F32, tag="p_big")
            for ns in (slice(0, 512), slice(512, H)):
                for kc in range(KC):
                    nc.tensor.matmul(p_a[:, ns], zsT[:, kc, :],
                                     wfzT[:, kc, ns],
                                     start=(kc == 0), stop=(kc == KC - 1))
            ab = spool.tile([128, H], BF16, tag="ab")
            nc.scalar.copy(ab[:], p_a[:])
            if b == 0:
                dump("d_ab", ab[:])

            # ---- nbr^T per h-chunk packed into one PSUM tile ----
            p_n = p_big.tile([128, H], F32, tag="p_big")
            for hc in range(KC):
                ns = slice(hc * 128, (hc + 1) * 128)
                nc.tensor.matmul(p_n[:, ns], ab[:, ns], attnT[:],
                                 start=True, stop=False)
                nc.tensor.matmul(p_n[:, ns], wfeT[:, ns], dT[:],
                                 start=False, stop=True)
            nbrT = spool.tile([128, KC, 128], BF16, tag="nbrT")
            nc.scalar.copy(nbrT[:], p_n[:])
            if b == 0:
                dump("d_nbrT", nbrT[:])

            # ---- temp = nbr @ WhN^T + zs @ WhZ^T ----
            p_t = p_big.tile([128, H], F32, tag="p_big")
            for ns in (slice(0, 512), slice(512, H)):
                for kc in range(KC):
                    nc.tensor.matmul(p_t[:, ns], nbrT[:, kc, :],
                                     whnT[:, kc, ns],
                                     start=(kc == 0), stop=False)
                for kc in range(KC):
                    nc.tensor.matmul(p_t[:, ns], zsT[:, kc, :],
                                     whzT[:, kc, ns],
                                     start=False, stop=(kc == KC - 1))
            tempb = opool.tile([128, H], F32, tag="tempb")
            nc.scalar.copy(tempb[:], p_t[:])
            if b == 0:
                dump("d_tempb", tempb[:])

            # ---- upd mask: span-row (host) -> column via rank-1; & any_j ----
            p_v = p_tr.tile([128, 384], F32, tag="p_tr")
            nc.tensor.matmul(p_v[:, 0:1], vrow4[0:1, b, :], ones_f[0:1, 0:1],
                             start=True, stop=True)
            anynb = spool.tile([128, 1], F32, tag="anynb")
            nc.vector.tensor_reduce(anynb[:], adjt[:], axis=AX.X, op=OP.max)
            upd = spool.tile([128, 1], F32, tag="upd")
            nc.vector.tensor_tensor(upd[:], p_v[:, 0:1], anynb[:], op=OP.mult)
            if b == 0:
                dump("d_upd", upd[:])

            # ---- blend + rolled store ----
            tdiff = opool.tile([128, H], F32, tag="tdiff")
            nc.gpsimd.tensor_tensor(tdiff[:], tempb[:], bertS[:],
                                    op=OP.subtract)
            outt = opool.tile([128, H], F32, tag="outt")
            nc.vector.scalar_tensor_tensor(
                outt[:], tdiff[:], upd[:], bertS[:], op0=OP.mult, op1=OP.add)
            nc.sync.dma_start(out[b, 1:128, :], outt[0:127, :])
            nc.sync.dma_start(out[b, 0:1, :], outt[127:128, :])


def _get_nc():
    if "nc" not in _CACHED:
        _CACHED["nc"] = _build()
    return _CACHED["nc"]


def _chunkT(w):
    """W [rows, K] -> W^T chunk-major [128, K//128, rows] (lhsT layout)."""
    rows, k = w.shape
    return np.ascontiguousarray(
        w.T.reshape(k // 128, 128, rows).transpose(1, 0, 2))


def _prep_in_maps(bert_hidden_states, dep_type_adj, deprel_adj,
                  asp_start, asp_end, Wz, bz, wa, ba, Wf, Wh):
    bf = ml_dtypes.bfloat16
    bert = np.ascontiguousarray(np.asarray(bert_hidden_states, np.float32))
    dep = np.asarray(dep_type_adj, np.float32).astype(bf)
    adjf = np.ascontiguousarray(np.asarray(deprel_adj).astype(np.float32))
    # bertS^T chunk-major per batch: rows shifted by one (the z-roll)
    bs = np.roll(bert, -1, axis=1)
    bertsT = np.ascontiguousarray(
        bs.transpose(0, 2, 1).reshape(B, KC, 128, L).transpose(0, 2, 1, 3)
    ).astype(bf)
    pos = np.arange(L, dtype=np.float32)
    s_ = np.asarray(asp_start).astype(np.float32)[:, None]
    e_ = np.asarray(asp_end).astype(np.float32)[:, None]
    vrow_full = ((pos[None, :] >= s_) & (pos[None, :] <= e_)).astype(np.float32)

    Wz = np.asarray(Wz, np.float32)
    Wf = np.asarray(Wf, np.float32)
    Wh = np.asarray(Wh, np.float32)
    wa_f = np.asarray(wa, np.float32)
    wzT = _chunkT(Wz).astype(bf)
    wfzT = _chunkT(Wf[:, :H]).astype(bf)
    whnT = _chunkT(Wh[:, :H]).astype(bf)
    whzT = _chunkT(Wh[:, H:]).astype(bf)
    wfeT = np.ascontiguousarray(Wf[:, H:].T).astype(bf)
    w2T = _chunkT(wa_f[:2 * H].reshape(2, H)).astype(bf)
    bzb = np.asarray(bz, np.float32)[None, :].astype(bf)
    waeb = wa_f[2 * H:][None, :].astype(bf)
    bab = np.asarray(ba, np.float32).reshape(1, 1)

    in_maps = []
    for c in range(NCORES):
        s = slice(c * PB, (c + 1) * PB)
        in_maps.append(dict(
            bert=bert[s], bertsT=np.ascontiguousarray(bertsT[s]),
            dep=dep[s], adjf=adjf[s],
            vrow=np.ascontiguousarray(vrow_full[s][None, :, :]),
            wzT=wzT, wfzT=wfzT, whnT=whnT, whzT=whzT, wfeT=wfeT, w2T=w2T,
            bzt=bzb, wae=waeb, bat=bab,
        ))
    return in_maps


def kernel(bert_hidden_states, dep_type_adj, deprel_adj, asp_start, asp_end,
           Wz, bz, wa, ba, Wf, Wh):
    from concourse.bass_utils import run_bass_kernel_spmd

    in_maps = _prep_in_maps(bert_hidden_states, dep_type_adj, deprel_adj,
                            asp_start, asp_end, Wz, bz, wa, ba, Wf, Wh)
    nc = _get_nc()
    res = run_bass_kernel_spmd(nc, in_maps, core_ids=list(range(NCORES)),
                               trace=bool(_CACHED.get("trace")),
                               tmpdir=_CACHED.get("trace_tmpdir"))
    _CACHED["last_results"] = res
    outs = [res.results[c]["out"] for c in range(NCORES)]
    return np.concatenate(outs, axis=0).astype(np.float32)



# revision 27
# speedup vs baseline: 1.1972x; 1.0706x over previous
"""Trainium2 Bass kernel for AspectNeighborAttention (gnn_message_passing).

Pure data-parallel over batch: 32 batches -> 8 NeuronCores x 4 batches.
All weights replicated, host-converted to bf16 and host-PRE-TRANSPOSED into
the chunk-major [128, KC, *] lhsT/rhs layouts the TensorEngine wants, so the
device does plain contiguous DMAs only. dep is host-bf16 (halves HBM traffic).

Per-core dataflow for each batch b (L=128 tokens, H=768, E=64, KC=6):
  zs^T   = Wz @ bertS^T + bz            (PE, bf16, packed PSUM groups)
  s_i,s_j= [wa_i;wa_j] @ zs^T           (PE, packed [1,128] regions)
  s_e    = reduce_e(dep * wa_e)         (DVE bf16 2x passes)
  score  = lrelu(s_i + s_j + s_e + ba)  (PE rank-1 bcast + DVE + ACT)
  attn   = mask * softmax(...)          (additive-shift masking, exp on ACT)
  D      = reduce_j(attn * dep)         (mult split Pool/DVE, bf16 2x reduce)
  nbr^T  = per-h-chunk matmuls from A/attn^T/D^T   (PE)
  temp   = nbr @ WhN^T + zs @ WhZ^T     (PE)
  out    = upd ? temp : bert            (blend, row-rolled DMA store)

The roll(z,-1)/roll(out,+1) pair is handled purely with shifted-row DMAs.
"""

import sys

for _p in ("/opt/trn_rl_repo",):
    if _p not in sys.path:
        sys.path.insert(0, _p)

import os
import numpy as np
import ml_dtypes

import concourse.bass as bass
import concourse.bacc as bacc_mod
import concourse.mybir as mybir
import concourse.tile as tile
from concourse.masks import make_identity

B, L, H, E = 32, 128, 768, 64
NCORES = 8
PB = B // NCORES  # batches per core
KC = H // 128     # 6 k-chunks
F32 = mybir.dt.float32
BF16 = mybir.dt.bfloat16
AF = mybir.ActivationFunctionType
OP = mybir.AluOpType
AX = mybir.AxisListType
MASK_SHIFT = 10000.0  # additive mask offset (see score masking)

_CACHED = {}

CFG = dict(
    dep_bufs=int(os.environ.get("K_DEP_BUFS", 2)),
    ttmp_bufs=int(os.environ.get("K_TTMP_BUFS", 3)),
    spool_bufs=int(os.environ.get("K_SPOOL_BUFS", 3)),
    opool_bufs=int(os.environ.get("K_OPOOL_BUFS", 2)),
    ptr_bufs=int(os.environ.get("K_PTR_BUFS", 2)),
    pbig_bufs=int(os.environ.get("K_PBIG_BUFS", 2)),
    jpd=int(os.environ.get("K_JPD", 48)),  # D-mult j-split: [0,jpd) Pool
    jp1=int(os.environ.get("K_JP1", 56)),  # s_e-mult j-split: [0,jp1) Pool
)


def _build(debug=False):
    nc = bacc_mod.Bacc("TRN2", target_bir_lowering=False, debug=False,
                       num_devices=NCORES)

    bert = nc.dram_tensor("berts", [PB, L, H], F32, kind="ExternalInput")
    bertsT = nc.dram_tensor("bertsT", [PB, 128, KC, 128], BF16,
                            kind="ExternalInput")
    dep = nc.dram_tensor("dep", [PB, L, L, E], BF16, kind="ExternalInput")
    adjf = nc.dram_tensor("adjf", [PB, L, L], F32, kind="ExternalInput")
    vrow = nc.dram_tensor("vrow", [1, PB, 128], F32, kind="ExternalInput")
    wzT_d = nc.dram_tensor("wzT", [128, KC, H], BF16, kind="ExternalInput")
    wfzT_d = nc.dram_tensor("wfzT", [128, KC, H], BF16, kind="ExternalInput")
    whnT_d = nc.dram_tensor("whnT", [128, KC, H], BF16, kind="ExternalInput")
    whzT_d = nc.dram_tensor("whzT", [128, KC, H], BF16, kind="ExternalInput")
    wfeT_d = nc.dram_tensor("wfeT", [E, H], BF16, kind="ExternalInput")
    w2T_d = nc.dram_tensor("w2T", [128, KC, 2], BF16, kind="ExternalInput")
    bzt = nc.dram_tensor("bzt", [1, H], BF16, kind="ExternalInput")
    wae = nc.dram_tensor("wae", [1, E], BF16, kind="ExternalInput")
    bat = nc.dram_tensor("bat", [1, 1], F32, kind="ExternalInput")
    out = nc.dram_tensor("out", [PB, L, H], F32, kind="ExternalOutput")

    dbg = {}
    if debug:
        for nm, shape, dt in [
            ("d_zsT", [128, KC, 128], BF16), ("d_si", [1, 128], F32),
            ("d_sjb", [1, 128], F32), ("d_se", [128, L], BF16),
            ("d_masked", [128, L], F32), ("d_attn", [128, L], BF16),
            ("d_dvec", [128, E], BF16), ("d_ab", [128, H], BF16),
            ("d_nbrT", [128, KC, 128], BF16), ("d_tempb", [128, H], F32),
            ("d_upd", [128, 1], F32), ("d_scb", [128, 128], F32),
        ]:
            dbg[nm] = nc.dram_tensor(nm, shape, dt, kind="ExternalOutput")
    with tile.TileContext(nc) as tc:
        with nc.allow_low_precision("bf16 softmax/D path, 2e-2 rel-err gate"):
            _body(tc, nc, bert, bertsT, dep, adjf, vrow, wzT_d, wfzT_d,
                  whnT_d, whzT_d, wfeT_d, w2T_d, bzt, wae, bat, out, dbg)
    nc.compile()
    return nc


def _body(tc, nc, bert, bertsT, dep, adjf, vrow, wzT_d, wfzT_d,
          whnT_d, whzT_d, wfeT_d, w2T_d, bzt, wae, bat, out, dbg=None):
    def dump(name, ap):
        if dbg and name in dbg:
            nc.sync.dma_start(dbg[name][...], ap)
    import contextlib
    cfg = CFG
    JPD = cfg["jpd"]
    JP1 = cfg["jp1"]
    ctx = contextlib.ExitStack()
    with ctx:
        wpool = ctx.enter_context(tc.tile_pool(name="weights", bufs=1))
        dpool = ctx.enter_context(
            tc.tile_pool(name="dep", bufs=cfg["dep_bufs"]))
        tpool = ctx.enter_context(
            tc.tile_pool(name="ttmp", bufs=cfg["ttmp_bufs"]))
        spool = ctx.enter_context(
            tc.tile_pool(name="small", bufs=cfg["spool_bufs"]))
        opool = ctx.enter_context(
            tc.tile_pool(name="outp", bufs=cfg["opool_bufs"]))
        p_tr = ctx.enter_context(
            tc.tile_pool(name="p_tr", bufs=cfg["ptr_bufs"], space="PSUM"))
        p_big = ctx.enter_context(
            tc.tile_pool(name="p_big", bufs=cfg["pbig_bufs"], space="PSUM"))

        # ---------------- one-time setup (plain DMAs only) ----------------
        wzT = wpool.tile([128, KC, H], BF16, tag="wzT")
        nc.sync.dma_start(wzT[:], wzT_d[...])
        wfzT = wpool.tile([128, KC, H], BF16, tag="wfzT")
        nc.sync.dma_start(wfzT[:], wfzT_d[...])
        whnT = wpool.tile([128, KC, H], BF16, tag="whnT")
        nc.sync.dma_start(whnT[:], whnT_d[...])
        whzT = wpool.tile([128, KC, H], BF16, tag="whzT")
        nc.sync.dma_start(whzT[:], whzT_d[...])
        wfeT = wpool.tile([E, H], BF16, tag="wfeT")
        nc.sync.dma_start(wfeT[:], wfeT_d[...])
        w2T = wpool.tile([128, KC, 2], BF16, tag="w2T")
        nc.sync.dma_start(w2T[:], w2T_d[...])
        bzr = wpool.tile([1, H], BF16, tag="bzr")
        nc.sync.dma_start(bzr[:], bzt[:, :])
        waer = wpool.tile([1, E], BF16, tag="waer")
        nc.sync.dma_start(waer[:], wae[:, :])
        bar = wpool.tile([1, 1], F32, tag="bar")
        nc.sync.dma_start(bar[:], bat[:, :])
        vrow4 = wpool.tile([1, PB, 128], F32, tag="vrow4")
        nc.sync.dma_start(vrow4[:], vrow[:, :, :])

        ones_f = wpool.tile([1, 128], F32, tag="ones_f")
        nc.gpsimd.memset(ones_f[:], 1.0)
        ones_b = wpool.tile([1, 128], BF16, tag="ones_b")
        nc.gpsimd.memset(ones_b[:], 1.0)
        id_bf = wpool.tile([128, 128], BF16, tag="id_bf")
        make_identity(nc, id_bf[:])

        # wa_e broadcast to all partitions via rank-1 matmul
        p_wae = p_tr.tile([128, 384], F32, tag="p_tr")
        nc.tensor.matmul(p_wae[:, 0:E], ones_b[:], waer[:],
                         start=True, stop=True)
        wae_bc = wpool.tile([128, E], BF16, tag="wae_bc")
        nc.scalar.copy(wae_bc[:], p_wae[:, 0:E])
        # dense [128, L, E] replica so the s_e mult runs in 2x dense mode
        wae_rep = wpool.tile([128, L, E], BF16, tag="wae_rep")
        nc.vector.tensor_copy(
            wae_rep[:], wae_bc[:].unsqueeze(1).broadcast_to([128, L, E]))

        # ---------------- per-batch pipeline ----------------
        for b in range(PB):
            # bertS: rows shifted by one token (z roll); f32 exact for blend
            bertS = spool.tile([128, H], F32, tag="bertS")
            nc.sync.dma_start(bertS[:], bert[b, :, :])
            bertST = spool.tile([128, KC, 128], BF16, tag="bertST")
            nc.sync.dma_start(bertST[:], bertsT[b, :, :, :])

            dept = dpool.tile([128, L, E], BF16, tag="dept")
            nc.sync.dma_start(dept[:, 0:64, :], dep[b, :, 0:64, :])
            nc.sync.dma_start(dept[:, 64:L, :], dep[b, :, 64:L, :])
            adjt = spool.tile([128, L], F32, tag="adjt")
            nc.sync.dma_start(adjt[:], adjf[b, :, :])

            # ---- zs^T = Wz @ bertS^T + bz: 6 groups packed in one PSUM ----
            p_z = p_big.tile([128, H], F32, tag="p_big")
            for hc in range(KC):
                ns = slice(hc * 128, (hc + 1) * 128)
                for kc in range(KC):
                    nc.tensor.matmul(p_z[:, ns], wzT[:, kc, ns],
                                     bertST[:, kc, :],
                                     start=(kc == 0), stop=False)
                nc.tensor.matmul(p_z[:, ns], bzr[0:1, ns], ones_b[:],
                                 start=False, stop=True)
            zsT = spool.tile([128, KC, 128], BF16, tag="zsT")
            nc.vector.tensor_copy(zsT[:], p_z[:])
            if b == 0:
                dump("d_zsT", zsT[:])

            # ---- s_i, s_j, score-base packed into one p_tr tile ----
            p_s3 = p_tr.tile([128, 384], F32, tag="p_tr")
            for kc in range(KC):
                nc.tensor.matmul(p_s3[0:1, 0:128], w2T[:, kc, 0:1],
                                 zsT[:, kc, :],
                                 start=(kc == 0), stop=(kc == KC - 1))
            for kc in range(KC):
                nc.tensor.matmul(p_s3[0:1, 128:256], w2T[:, kc, 1:2],
                                 zsT[:, kc, :],
                                 start=(kc == 0), stop=(kc == KC - 1))
            si_row = spool.tile([1, 128], F32, tag="si_row")
            nc.scalar.copy(si_row[:], p_s3[0:1, 0:128])
            sjb = spool.tile([1, 128], F32, tag="sjb")
            nc.vector.tensor_scalar(sjb[:], p_s3[0:1, 128:256], bar[0:1, 0:1],
                                    None, op0=OP.add)
            # score base: s_i (row-bcast) + (s_j + ba) (col-bcast)
            nc.tensor.matmul(p_s3[:, 256:384], si_row[:], ones_f[:],
                             start=True, stop=False)
            nc.tensor.matmul(p_s3[:, 256:384], ones_f[:], sjb[:],
                             start=False, stop=True)
            if b == 0:
                dump("d_si", si_row[:])
                dump("d_sjb", sjb[:])

            # ---- s_e = reduce_e(dep * wa_e); dense mult split Pool/DVE ----
            tmp1 = tpool.tile([128, L, E], BF16, tag="ttmp")
            nc.gpsimd.tensor_tensor(
                tmp1[:, 0:JP1, :], dept[:, 0:JP1, :],
                wae_rep[:, 0:JP1, :], op=OP.mult)
            nc.vector.tensor_tensor(
                tmp1[:, JP1:L, :], dept[:, JP1:L, :],
                wae_rep[:, JP1:L, :], op=OP.mult)
            se = spool.tile([128, L], BF16, tag="se")
            nc.vector.tensor_reduce(se[:], tmp1[:], axis=AX.X, op=OP.add)
            if b == 0:
                dump("d_se", se[:])
                scb_s = spool.tile([128, 128], F32, tag="scb_s")
                nc.vector.tensor_copy(scb_s[:], p_s3[:, 256:384])
                dump("d_scb", scb_s[:])

            # ---- score = lrelu(se + base); masked = (score+C)*m ----
            sadd = spool.tile([128, L], F32, tag="sadd")
            nc.vector.tensor_tensor(sadd[:], se[:], p_s3[:, 256:384],
                                    op=OP.add)
            # lrelu(x) = max(0.01*x, x) on DVE (keeps ACT table = Exp only)
            score = spool.tile([128, L], F32, tag="score")
            nc.vector.scalar_tensor_tensor(
                score[:], sadd[:], 0.01, sadd[:], op0=OP.mult, op1=OP.max)
            masked = spool.tile([128, L], F32, tag="masked")
            nc.vector.scalar_tensor_tensor(
                masked[:], score[:], MASK_SHIFT, adjt[:],
                op0=OP.add, op1=OP.mult)
            if b == 0:
                dump("d_masked", masked[:])

            # ---- softmax over j (free axis); attn emitted directly bf16 ----
            mxn = spool.tile([128, 1], F32, tag="mxn")
            nc.vector.tensor_reduce(mxn[:], masked[:], axis=AX.X, op=OP.max,
                                    negate=True)
            ex = spool.tile([128, L], F32, tag="ex")
            sumex = spool.tile([128, 1], F32, tag="sumex")
            nc.scalar.activation(ex[:], masked[:], AF.Exp, bias=mxn[:],
                                 scale=1.0, accum_out=sumex[:])
            rec = spool.tile([128, 1], F32, tag="rec")
            nc.vector.reciprocal(rec[:], sumex[:])
            attnb = spool.tile([128, L], BF16, tag="attnb")
            nc.vector.scalar_tensor_tensor(
                attnb[:], ex[:], rec[:], adjt[:], op0=OP.mult, op1=OP.mult)
            if b == 0:
                dump("d_attn", attnb[:])

            # ---- D = reduce_j(attn * dep) in natural [i,j,e] layout ----
            # mult: contiguous, attn broadcast along inner e; split Pool/DVE
            tmp2 = tpool.tile([128, L, E], BF16, tag="ttmp")
            nc.gpsimd.tensor_tensor(
                tmp2[:, 0:JPD, :], dept[:, 0:JPD, :],
                attnb[:, 0:JPD].unsqueeze(2).broadcast_to([128, JPD, E]),
                op=OP.mult)
            nc.vector.tensor_tensor(
                tmp2[:, JPD:L, :], dept[:, JPD:L, :],
                attnb[:, JPD:L].unsqueeze(2).broadcast_to([128, L - JPD, E]),
                op=OP.mult)
            # pairwise tree over j, ping-ponged through tmp3 (no aliasing):
            # Pool reduces j[0:64) -> row 0, DVE j[64:128) -> row 64.
            tmp3 = tpool.tile([128, L, E], BF16, tag="ttmp")
            src, dst = tmp2, tmp3
            h = 32
            while h >= 1:
                nc.gpsimd.tensor_tensor(
                    dst[:, 0:h, :], src[:, 0:h, :], src[:, h:2 * h, :],
                    op=OP.add)
                nc.vector.tensor_tensor(
                    dst[:, 64:64 + h, :], src[:, 64:64 + h, :],
                    src[:, 64 + h:64 + 2 * h, :], op=OP.add)
                src, dst = dst, src
                h //= 2
            dvb = spool.tile([128, E], BF16, tag="dvb")
            nc.vector.tensor_tensor(dvb[:], src[:, 0, :], src[:, 64, :],
                                    op=OP.add)
            if b == 0:
                dump("d_dvec", dvb[:])

            # attn^T and D^T via PE transposes into one packed PSUM tile
            p_ad = p_tr.tile([128, 256], BF16, tag="p_tr")
            nc.tensor.transpose(p_ad[:, 0:128], attnb[:], id_bf[:])
            nc.tensor.transpose(p_ad[0:E, 128:256], dvb[:], id_bf[:])
            attnT = spool.tile([128, 128], BF16, tag="attnT")
            nc.vector.tensor_copy(attnT[:], p_ad[:, 0:128])
            dT = spool.tile([E, 128], BF16, tag="dT")
            nc.vector.tensor_copy(dT[:], p_ad[0:E, 128:256])

            # ---- A = zs @ WfZ^T  ([j, h], bf16) ----
            p_a = p_big.tile([128, H], F32, tag="p_big")
            for ns in (slice(0, 512), slice(512, H)):
                for kc in range(KC):
                    nc.tensor.matmul(p_a[:, ns], zsT[:, kc, :],
                                     wfzT[:, kc, ns],
                                     start=(kc == 0), stop=(kc == KC - 1))
            ab = spool.tile([128, H], BF16, tag="ab")
            nc.vector.tensor_copy(ab[:], p_a[:])
            if b == 0:
                dump("d_ab", ab[:])

            # ---- nbr^T per h-chunk packed into one PSUM tile ----
            p_n = p_big.tile([128, H], F32, tag="p_big")
            for hc in range(KC):
                ns = slice(hc * 128, (hc + 1) * 128)
                nc.tensor.matmul(p_n[:, ns], ab[:, ns], attnT[:],
                                 start=True, stop=False)
                nc.tensor.matmul(p_n[:, ns], wfeT[:, ns], dT[:],
                                 start=False, stop=True)
            nbrT = spool.tile([128, KC, 128], BF16, tag="nbrT")
            nc.vector.tensor_copy(nbrT[:], p_n[:])
            if b == 0:
                dump("d_nbrT", nbrT[:])

            # ---- temp = nbr @ WhN^T + zs @ WhZ^T ----
            p_t = p_big.tile([128, H], F32, tag="p_big")
            for ns in (slice(0, 512), slice(512, H)):
                for kc in range(KC):
                    nc.tensor.matmul(p_t[:, ns], nbrT[:, kc, :],
                                     whnT[:, kc, ns],
                                     start=(kc == 0), stop=False)
                for kc in range(KC):
                    nc.tensor.matmul(p_t[:, ns], zsT[:, kc, :],
                                     whzT[:, kc, ns],
                                     start=False, stop=(kc == KC - 1))
            tempb = opool.tile([128, H], F32, tag="tempb")
            nc.vector.tensor_copy(tempb[:], p_t[:])
            if b == 0:
                dump("d_tempb", tempb[:])

            # ---- upd mask: span-row (host) -> column via rank-1; & any_j ----
            p_v = p_tr.tile([128, 384], F32, tag="p_tr")
            nc.tensor.matmul(p_v[:, 0:1], vrow4[0:1, b, :], ones_f[0:1, 0:1],
                             start=True, stop=True)
            anynb = spool.tile([128, 1], F32, tag="anynb")
            nc.vector.tensor_reduce(anynb[:], adjt[:], axis=AX.X, op=OP.max)
            upd = spool.tile([128, 1], F32, tag="upd")
            nc.vector.tensor_tensor(upd[:], p_v[:, 0:1], anynb[:], op=OP.mult)
            if b == 0:
                dump("d_upd", upd[:])

            # ---- blend + rolled store ----
            tdiff = opool.tile([128, H], F32, tag="tdiff")
            nc.gpsimd.tensor_tensor(tdiff[:], tempb[:], bertS[:],
                                    op=OP.subtract)
            outt = opool.tile([128, H], F32, tag="outt")
            nc.vector.scalar_tensor_tensor(
                outt[:], tdiff[:], upd[:], bertS[:], op0=OP.mult, op1=OP.add)
            nc.sync.dma_start(out[b, 1:128, :], outt[0:127, :])
            nc.sync.dma_start(out[b, 0:1, :], outt[127:128, :])


def _get_nc():
    if "nc" not in _CACHED:
        _CACHED["nc"] = _build()
    return _CACHED["nc"]


def _chunkT(w):
    """W [rows, K] -> W^T chunk-major [128, K//128, rows] (lhsT layout)."""
    rows, k = w.shape
    return np.ascontiguousarray(
        w.T.reshape(k // 128, 128, rows).transpose(1, 0, 2))


def _prep_in_maps(bert_hidden_states, dep_type_adj, deprel_adj,
                  asp_start, asp_end, Wz, bz, wa, ba, Wf, Wh):
    bf = ml_dtypes.bfloat16
    bert = np.ascontiguousarray(np.asarray(bert_hidden_states, np.float32))
    dep = np.asarray(dep_type_adj, np.float32).astype(bf)
    adjf = np.ascontiguousarray(np.asarray(deprel_adj).astype(np.float32))
    # bertS^T chunk-major per batch: rows shifted by one (the z-roll)
    bs = np.ascontiguousarray(np.roll(bert, -1, axis=1))
    bertsT = np.ascontiguousarray(
        bs.transpose(0, 2, 1).reshape(B, KC, 128, L).transpose(0, 2, 1, 3)
    ).astype(bf)
    pos = np.arange(L, dtype=np.float32)
    s_ = np.asarray(asp_start).astype(np.float32)[:, None]
    e_ = np.asarray(asp_end).astype(np.float32)[:, None]
    vrow_full = ((pos[None, :] >= s_) & (pos[None, :] <= e_)).astype(np.float32)

    Wz = np.asarray(Wz, np.float32)
    Wf = np.asarray(Wf, np.float32)
    Wh = np.asarray(Wh, np.float32)
    wa_f = np.asarray(wa, np.float32)
    wzT = _chunkT(Wz).astype(bf)
    wfzT = _chunkT(Wf[:, :H]).astype(bf)
    whnT = _chunkT(Wh[:, :H]).astype(bf)
    whzT = _chunkT(Wh[:, H:]).astype(bf)
    wfeT = np.ascontiguousarray(Wf[:, H:].T).astype(bf)
    w2T = _chunkT(wa_f[:2 * H].reshape(2, H)).astype(bf)
    bzb = np.asarray(bz, np.float32)[None, :].astype(bf)
    waeb = wa_f[2 * H:][None, :].astype(bf)
    bab = np.asarray(ba, np.float32).reshape(1, 1)

    in_maps = []
    for c in range(NCORES):
        s = slice(c * PB, (c + 1) * PB)
        in_maps.append(dict(
            berts=bs[s], bertsT=np.ascontiguousarray(bertsT[s]),
            dep=dep[s], adjf=adjf[s],
            vrow=np.ascontiguousarray(vrow_full[s][None, :, :]),
            wzT=wzT, wfzT=wfzT, whnT=whnT, whzT=whzT, wfeT=wfeT, w2T=w2T,
            bzt=bzb, wae=waeb, bat=bab,
        ))
    return in_maps


def kernel(bert_hidden_states, dep_type_adj, deprel_adj, asp_start, asp_end,
           Wz, bz, wa, ba, Wf, Wh):
    from concourse.bass_utils import run_bass_kernel_spmd

    in_maps = _prep_in_maps(bert_hidden_states, dep_type_adj, deprel_adj,
                            asp_start, asp_end, Wz, bz, wa, ba, Wf, Wh)
    nc = _get_nc()
    res = run_bass_kernel_spmd(nc, in_maps, core_ids=list(range(NCORES)),
                               trace=bool(_CACHED.get("trace")),
                               tmpdir=_CACHED.get("trace_tmpdir"))
    _CACHED["last_results"] = res
    outs = [res.results[c]["out"] for c in range(NCORES)]
    return np.concatenate(outs, axis=0).astype(np.float32)



# revision 29
# speedup vs baseline: 1.2904x; 1.0779x over previous
"""Trainium2 Bass kernel for AspectNeighborAttention (gnn_message_passing).

Pure data-parallel over batch: 32 batches -> 8 NeuronCores x 4 batches.
All weights replicated, host-converted to bf16 and host-PRE-TRANSPOSED into
the chunk-major [128, KC, *] lhsT/rhs layouts the TensorEngine wants, so the
device does plain contiguous DMAs only. dep is host-bf16 (halves HBM traffic).

Per-core dataflow for each batch b (L=128 tokens, H=768, E=64, KC=6):
  zs^T   = Wz @ bertS^T + bz            (PE, bf16, packed PSUM groups)
  s_i,s_j= [wa_i;wa_j] @ zs^T           (PE, packed [1,128] regions)
  s_e    = reduce_e(dep * wa_e)         (DVE bf16 2x passes)
  score  = lrelu(s_i + s_j + s_e + ba)  (PE rank-1 bcast + DVE + ACT)
  attn   = mask * softmax(...)          (additive-shift masking, exp on ACT)
  D      = reduce_j(attn * dep)         (mult split Pool/DVE, bf16 2x reduce)
  nbr^T  = per-h-chunk matmuls from A/attn^T/D^T   (PE)
  temp   = nbr @ WhN^T + zs @ WhZ^T     (PE)
  out    = upd ? temp : bert            (blend, row-rolled DMA store)

The roll(z,-1)/roll(out,+1) pair is handled purely with shifted-row DMAs.
"""

import sys

for _p in ("/opt/trn_rl_repo",):
    if _p not in sys.path:
        sys.path.insert(0, _p)

import os
import numpy as np
import ml_dtypes

import concourse.bass as bass
import concourse.bacc as bacc_mod
import concourse.mybir as mybir
import concourse.tile as tile
from concourse.masks import make_identity

B, L, H, E = 32, 128, 768, 64
NCORES = 8
PB = B // NCORES  # batches per core
KC = H // 128     # 6 k-chunks
F32 = mybir.dt.float32
BF16 = mybir.dt.bfloat16
AF = mybir.ActivationFunctionType
OP = mybir.AluOpType
AX = mybir.AxisListType
MASK_SHIFT = 10000.0  # additive mask offset (see score masking)

_CACHED = {}

CFG = dict(
    dep_bufs=int(os.environ.get("K_DEP_BUFS", 2)),
    ttmp_bufs=int(os.environ.get("K_TTMP_BUFS", 3)),
    spool_bufs=int(os.environ.get("K_SPOOL_BUFS", 3)),
    opool_bufs=int(os.environ.get("K_OPOOL_BUFS", 2)),
    ptr_bufs=int(os.environ.get("K_PTR_BUFS", 2)),
    pbig_bufs=int(os.environ.get("K_PBIG_BUFS", 2)),
    jpd=int(os.environ.get("K_JPD", 36)),  # D-mult j-split: [0,jpd) Pool
    jp1=int(os.environ.get("K_JP1", 38)),  # s_e-mult j-split: [0,jp1) Pool
)


def _build(debug=False):
    nc = bacc_mod.Bacc("TRN2", target_bir_lowering=False, debug=False,
                       num_devices=NCORES)

    bert = nc.dram_tensor("berts", [PB, L, H], F32, kind="ExternalInput")
    bertsT = nc.dram_tensor("bertsT", [PB, 128, KC, 128], BF16,
                            kind="ExternalInput")
    dep = nc.dram_tensor("dep", [PB, L, L, E], BF16, kind="ExternalInput")
    adjf = nc.dram_tensor("adjf", [PB, L, L], F32, kind="ExternalInput")
    vrow = nc.dram_tensor("vrow", [1, PB, 128], F32, kind="ExternalInput")
    wzT_d = nc.dram_tensor("wzT", [128, KC, H], BF16, kind="ExternalInput")
    wfzT_d = nc.dram_tensor("wfzT", [128, KC, H], BF16, kind="ExternalInput")
    whnT_d = nc.dram_tensor("whnT", [128, KC, H], BF16, kind="ExternalInput")
    whzT_d = nc.dram_tensor("whzT", [128, KC, H], BF16, kind="ExternalInput")
    wfeT_d = nc.dram_tensor("wfeT", [E, H], BF16, kind="ExternalInput")
    w2T_d = nc.dram_tensor("w2T", [128, KC, 2], BF16, kind="ExternalInput")
    bzt = nc.dram_tensor("bzt", [1, H], BF16, kind="ExternalInput")
    wae = nc.dram_tensor("wae", [1, E], BF16, kind="ExternalInput")
    bat = nc.dram_tensor("bat", [1, 1], F32, kind="ExternalInput")
    out = nc.dram_tensor("out", [PB, L, H], F32, kind="ExternalOutput")

    dbg = {}
    if debug:
        for nm, shape, dt in [
            ("d_zsT", [128, KC, 128], BF16), ("d_si", [1, 128], F32),
            ("d_sjb", [1, 128], F32), ("d_se", [128, L], BF16),
            ("d_masked", [128, L], F32), ("d_attn", [128, L], BF16),
            ("d_dvec", [128, E], BF16), ("d_ab", [128, H], BF16),
            ("d_nbrT", [128, KC, 128], BF16), ("d_tempb", [128, H], F32),
            ("d_upd", [128, 1], F32), ("d_scb", [128, 128], F32),
        ]:
            dbg[nm] = nc.dram_tensor(nm, shape, dt, kind="ExternalOutput")
    with tile.TileContext(nc) as tc:
        with nc.allow_low_precision("bf16 softmax/D path, 2e-2 rel-err gate"):
            _body(tc, nc, bert, bertsT, dep, adjf, vrow, wzT_d, wfzT_d,
                  whnT_d, whzT_d, wfeT_d, w2T_d, bzt, wae, bat, out, dbg)
    nc.compile()
    return nc


def _body(tc, nc, bert, bertsT, dep, adjf, vrow, wzT_d, wfzT_d,
          whnT_d, whzT_d, wfeT_d, w2T_d, bzt, wae, bat, out, dbg=None):
    def dump(name, ap):
        if dbg and name in dbg:
            nc.sync.dma_start(dbg[name][...], ap)
    import contextlib
    cfg = CFG
    JPD = cfg["jpd"]
    JP1 = cfg["jp1"]
    ctx = contextlib.ExitStack()
    with ctx:
        wpool = ctx.enter_context(tc.tile_pool(name="weights", bufs=1))
        dpool = ctx.enter_context(
            tc.tile_pool(name="dep", bufs=cfg["dep_bufs"]))
        tpool = ctx.enter_context(
            tc.tile_pool(name="ttmp", bufs=cfg["ttmp_bufs"]))
        spool = ctx.enter_context(
            tc.tile_pool(name="small", bufs=cfg["spool_bufs"]))
        opool = ctx.enter_context(
            tc.tile_pool(name="outp", bufs=cfg["opool_bufs"]))
        p_tr = ctx.enter_context(
            tc.tile_pool(name="p_tr", bufs=cfg["ptr_bufs"], space="PSUM"))
        p_big = ctx.enter_context(
            tc.tile_pool(name="p_big", bufs=cfg["pbig_bufs"], space="PSUM"))

        # ---------------- one-time setup (plain DMAs only) ----------------
        wzT = wpool.tile([128, KC, H], BF16, tag="wzT")
        nc.sync.dma_start(wzT[:], wzT_d[...])
        wfzT = wpool.tile([128, KC, H], BF16, tag="wfzT")
        nc.sync.dma_start(wfzT[:], wfzT_d[...])
        whnT = wpool.tile([128, KC, H], BF16, tag="whnT")
        nc.sync.dma_start(whnT[:], whnT_d[...])
        whzT = wpool.tile([128, KC, H], BF16, tag="whzT")
        nc.sync.dma_start(whzT[:], whzT_d[...])
        wfeT = wpool.tile([E, H], BF16, tag="wfeT")
        nc.sync.dma_start(wfeT[:], wfeT_d[...])
        w2T = wpool.tile([128, KC, 2], BF16, tag="w2T")
        nc.sync.dma_start(w2T[:], w2T_d[...])
        bzr = wpool.tile([1, H], BF16, tag="bzr")
        nc.sync.dma_start(bzr[:], bzt[:, :])
        waer = wpool.tile([1, E], BF16, tag="waer")
        nc.sync.dma_start(waer[:], wae[:, :])
        bar = wpool.tile([1, 1], F32, tag="bar")
        nc.sync.dma_start(bar[:], bat[:, :])
        vrow4 = wpool.tile([1, PB, 128], F32, tag="vrow4")
        nc.sync.dma_start(vrow4[:], vrow[:, :, :])

        ones_f = wpool.tile([1, 128], F32, tag="ones_f")
        nc.gpsimd.memset(ones_f[:], 1.0)
        ones_b = wpool.tile([1, 128], BF16, tag="ones_b")
        nc.gpsimd.memset(ones_b[:], 1.0)
        id_bf = wpool.tile([128, 128], BF16, tag="id_bf")
        make_identity(nc, id_bf[:])

        # wa_e broadcast to all partitions via rank-1 matmul
        p_wae = p_tr.tile([128, 512], F32, tag="p_tr")
        nc.tensor.matmul(p_wae[:, 0:E], ones_b[:], waer[:],
                         start=True, stop=True)
        wae_bc = wpool.tile([128, E], BF16, tag="wae_bc")
        nc.scalar.copy(wae_bc[:], p_wae[:, 0:E])
        # dense [128, L, E] replica so the s_e mult runs in 2x dense mode
        wae_rep = wpool.tile([128, L, E], BF16, tag="wae_rep")
        nc.vector.tensor_copy(
            wae_rep[:], wae_bc[:].unsqueeze(1).broadcast_to([128, L, E]))

        # ---------------- per-batch pipeline ----------------
        for b in range(PB):
            # bertS: rows shifted by one token (z roll); f32 exact for blend
            bertS = spool.tile([128, H], F32, tag="bertS")
            nc.sync.dma_start(bertS[:], bert[b, :, :])
            bertST = spool.tile([128, KC, 128], BF16, tag="bertST")
            nc.sync.dma_start(bertST[:], bertsT[b, :, :, :])

            dept = dpool.tile([128, L, E], BF16, tag="dept")
            nc.sync.dma_start(dept[:, 0:64, :], dep[b, :, 0:64, :])
            nc.sync.dma_start(dept[:, 64:L, :], dep[b, :, 64:L, :])
            adjt = spool.tile([128, L], F32, tag="adjt")
            nc.sync.dma_start(adjt[:], adjf[b, :, :])

            # ---- zs^T = Wz @ bertS^T + bz: 6 groups packed in one PSUM ----
            p_z = p_big.tile([128, H], F32, tag="p_big")
            for hc in range(KC):
                ns = slice(hc * 128, (hc + 1) * 128)
                for kc in range(KC):
                    nc.tensor.matmul(p_z[:, ns], wzT[:, kc, ns],
                                     bertST[:, kc, :],
                                     start=(kc == 0), stop=False)
                nc.tensor.matmul(p_z[:, ns], bzr[0:1, ns], ones_b[:],
                                 start=False, stop=True)
            zsT = spool.tile([128, KC, 128], BF16, tag="zsT")
            nc.scalar.copy(zsT[:], p_z[:])
            if b == 0:
                dump("d_zsT", zsT[:])

            # ---- s_i, s_j, score-base packed into one p_tr tile ----
            p_s3 = p_tr.tile([128, 512], F32, tag="p_tr")
            for kc in range(KC):
                nc.tensor.matmul(p_s3[0:1, 0:128], w2T[:, kc, 0:1],
                                 zsT[:, kc, :],
                                 start=(kc == 0), stop=(kc == KC - 1))
            for kc in range(KC):
                nc.tensor.matmul(p_s3[0:1, 128:256], w2T[:, kc, 1:2],
                                 zsT[:, kc, :],
                                 start=(kc == 0), stop=(kc == KC - 1))
            si_row = spool.tile([1, 128], F32, tag="si_row")
            nc.scalar.copy(si_row[:], p_s3[0:1, 0:128])
            sjb = spool.tile([1, 128], F32, tag="sjb")
            nc.vector.tensor_scalar(sjb[:], p_s3[0:1, 128:256], bar[0:1, 0:1],
                                    None, op0=OP.add)
            # si as a per-partition column; (s_j + ba) row-bcast to all rows
            nc.tensor.matmul(p_s3[:, 384:385], si_row[:], ones_f[0:1, 0:1],
                             start=True, stop=True)
            nc.tensor.matmul(p_s3[:, 256:384], ones_f[:], sjb[:],
                             start=True, stop=True)
            if b == 0:
                dump("d_si", si_row[:])
                dump("d_sjb", sjb[:])

            # ---- s_e = reduce_e(dep * wa_e); dense mult split Pool/DVE ----
            tmp1 = tpool.tile([128, L, E], BF16, tag="ttmp")
            nc.gpsimd.tensor_tensor(
                tmp1[:, 0:JP1, :], dept[:, 0:JP1, :],
                wae_rep[:, 0:JP1, :], op=OP.mult)
            nc.vector.tensor_tensor(
                tmp1[:, JP1:L, :], dept[:, JP1:L, :],
                wae_rep[:, JP1:L, :], op=OP.mult)
            se = spool.tile([128, L], BF16, tag="se")
            nc.vector.tensor_reduce(se[:], tmp1[:], axis=AX.X, op=OP.add)
            if b == 0:
                dump("d_se", se[:])
                scb_s = spool.tile([128, 128], F32, tag="scb_s")
                nc.vector.tensor_copy(scb_s[:], p_s3[:, 256:384])
                dump("d_scb", scb_s[:])

            # ---- score = lrelu(se + base); masked = (score+C)*m ----
            sadd = spool.tile([128, L], F32, tag="sadd")
            nc.vector.scalar_tensor_tensor(
                sadd[:], se[:], p_s3[:, 384:385], p_s3[:, 256:384],
                op0=OP.add, op1=OP.add)
            # lrelu(x) = max(0.01*x, x) on DVE (keeps ACT table = Exp only)
            score = spool.tile([128, L], F32, tag="score")
            nc.vector.scalar_tensor_tensor(
                score[:], sadd[:], 0.01, sadd[:], op0=OP.mult, op1=OP.max)
            masked = spool.tile([128, L], F32, tag="masked")
            nc.vector.scalar_tensor_tensor(
                masked[:], score[:], MASK_SHIFT, adjt[:],
                op0=OP.add, op1=OP.mult)
            if b == 0:
                dump("d_masked", masked[:])

            # ---- softmax over j (free axis); attn emitted directly bf16 ----
            mxn = spool.tile([128, 1], F32, tag="mxn")
            nc.vector.tensor_reduce(mxn[:], masked[:], axis=AX.X, op=OP.max,
                                    negate=True)
            ex = spool.tile([128, L], F32, tag="ex")
            sumex = spool.tile([128, 1], F32, tag="sumex")
            nc.scalar.activation(ex[:], masked[:], AF.Exp, bias=mxn[:],
                                 scale=1.0, accum_out=sumex[:])
            rec = spool.tile([128, 1], F32, tag="rec")
            nc.vector.reciprocal(rec[:], sumex[:])
            attnb = spool.tile([128, L], BF16, tag="attnb")
            nc.vector.scalar_tensor_tensor(
                attnb[:], ex[:], rec[:], adjt[:], op0=OP.mult, op1=OP.mult)
            if b == 0:
                dump("d_attn", attnb[:])

            # ---- D = reduce_j(attn * dep) in natural [i,j,e] layout ----
            # mult: contiguous, attn broadcast along inner e; split Pool/DVE
            tmp2 = tpool.tile([128, L, E], BF16, tag="ttmp")
            nc.gpsimd.tensor_tensor(
                tmp2[:, 0:JPD, :], dept[:, 0:JPD, :],
                attnb[:, 0:JPD].unsqueeze(2).broadcast_to([128, JPD, E]),
                op=OP.mult)
            nc.vector.tensor_tensor(
                tmp2[:, JPD:L, :], dept[:, JPD:L, :],
                attnb[:, JPD:L].unsqueeze(2).broadcast_to([128, L - JPD, E]),
                op=OP.mult)
            # pairwise tree over j, ping-ponged through tmp3 (no aliasing):
            # Pool reduces j[0:64) -> row 0, DVE j[64:128) -> row 64.
            tmp3 = tpool.tile([128, L, E], BF16, tag="ttmp")
            src, dst = tmp2, tmp3
            h = 32
            while h >= 1:
                nc.gpsimd.tensor_tensor(
                    dst[:, 0:h, :], src[:, 0:h, :], src[:, h:2 * h, :],
                    op=OP.add)
                nc.vector.tensor_tensor(
                    dst[:, 64:64 + h, :], src[:, 64:64 + h, :],
                    src[:, 64 + h:64 + 2 * h, :], op=OP.add)
                src, dst = dst, src
                h //= 2
            dvb = spool.tile([128, E], BF16, tag="dvb")
            nc.vector.tensor_tensor(dvb[:], src[:, 0, :], src[:, 64, :],
                                    op=OP.add)
            if b == 0:
                dump("d_dvec", dvb[:])

            # attn^T and D^T via PE transposes into one packed PSUM tile
            p_ad = p_tr.tile([128, 256], BF16, tag="p_tr")
            nc.tensor.transpose(p_ad[:, 0:128], attnb[:], id_bf[:])
            nc.tensor.transpose(p_ad[0:E, 128:256], dvb[:], id_bf[:])
            attnT = spool.tile([128, 128], BF16, tag="attnT")
            nc.scalar.copy(attnT[:], p_ad[:, 0:128])
            dT = spool.tile([E, 128], BF16, tag="dT")
            nc.scalar.copy(dT[:], p_ad[0:E, 128:256])

            # ---- A = zs @ WfZ^T  ([j, h], bf16) ----
            p_a = p_big.tile([128, H], F32, tag="p_big")
            for ns in (slice(0, 512), slice(512, H)):
                for kc in range(KC):
                    nc.tensor.matmul(p_a[:, ns], zsT[:, kc, :],
                                     wfzT[:, kc, ns],
                                     start=(kc == 0), stop=(kc == KC - 1))
            ab = spool.tile([128, H], BF16, tag="ab")
            nc.scalar.copy(ab[:], p_a[:])
            if b == 0:
                dump("d_ab", ab[:])

            # ---- nbr^T per h-chunk packed into one PSUM tile ----
            p_n = p_big.tile([128, H], F32, tag="p_big")
            for hc in range(KC):
                ns = slice(hc * 128, (hc + 1) * 128)
                nc.tensor.matmul(p_n[:, ns], ab[:, ns], attnT[:],
                                 start=True, stop=False)
                nc.tensor.matmul(p_n[:, ns], wfeT[:, ns], dT[:],
                                 start=False, stop=True)
            nbrT = spool.tile([128, KC, 128], BF16, tag="nbrT")
            nc.scalar.copy(nbrT[:], p_n[:])
            if b == 0:
                dump("d_nbrT", nbrT[:])

            # ---- temp = nbr @ WhN^T + zs @ WhZ^T ----
            p_t = p_big.tile([128, H], F32, tag="p_big")
            for ns in (slice(0, 512), slice(512, H)):
                for kc in range(KC):
                    nc.tensor.matmul(p_t[:, ns], nbrT[:, kc, :],
                                     whnT[:, kc, ns],
                                     start=(kc == 0), stop=False)
                for kc in range(KC):
                    nc.tensor.matmul(p_t[:, ns], zsT[:, kc, :],
                                     whzT[:, kc, ns],
                                     start=False, stop=(kc == KC - 1))
            tempb = opool.tile([128, H], F32, tag="tempb")
            nc.scalar.copy(tempb[:], p_t[:])
            if b == 0:
                dump("d_tempb", tempb[:])

            # ---- upd mask: (span & any-neighbor) host row -> column ----
            p_v = p_tr.tile([128, 512], F32, tag="p_tr")
            nc.tensor.matmul(p_v[:, 0:1], vrow4[0:1, b, :], ones_f[0:1, 0:1],
                             start=True, stop=True)

            # ---- blend + rolled store ----
            tdiff = opool.tile([128, H], F32, tag="tdiff")
            nc.gpsimd.tensor_tensor(tdiff[:], tempb[:], bertS[:],
                                    op=OP.subtract)
            outt = opool.tile([128, H], F32, tag="outt")
            nc.vector.scalar_tensor_tensor(
                outt[:], tdiff[:], p_v[:, 0:1], bertS[:],
                op0=OP.mult, op1=OP.add)
            nc.sync.dma_start(out[b, 1:128, :], outt[0:127, :])
            nc.sync.dma_start(out[b, 0:1, :], outt[127:128, :])


def _get_nc():
    if "nc" not in _CACHED:
        _CACHED["nc"] = _build()
    return _CACHED["nc"]


def _chunkT(w):
    """W [rows, K] -> W^T chunk-major [128, K//128, rows] (lhsT layout)."""
    rows, k = w.shape
    return np.ascontiguousarray(
        w.T.reshape(k // 128, 128, rows).transpose(1, 0, 2))


def _prep_in_maps(bert_hidden_states, dep_type_adj, deprel_adj,
                  asp_start, asp_end, Wz, bz, wa, ba, Wf, Wh):
    bf = ml_dtypes.bfloat16
    bert = np.ascontiguousarray(np.asarray(bert_hidden_states, np.float32))
    dep = np.asarray(dep_type_adj, np.float32).astype(bf)
    adjf = np.ascontiguousarray(np.asarray(deprel_adj).astype(np.float32))
    # bertS^T chunk-major per batch: rows shifted by one (the z-roll)
    bs = np.ascontiguousarray(np.roll(bert, -1, axis=1))
    bertsT = np.ascontiguousarray(
        bs.transpose(0, 2, 1).reshape(B, KC, 128, L).transpose(0, 2, 1, 3)
    ).astype(bf)
    pos = np.arange(L, dtype=np.float32)
    s_ = np.asarray(asp_start).astype(np.float32)[:, None]
    e_ = np.asarray(asp_end).astype(np.float32)[:, None]
    vrow_full = (((pos[None, :] >= s_) & (pos[None, :] <= e_))
                 & (np.asarray(deprel_adj) > 0).any(-1)).astype(np.float32)

    Wz = np.asarray(Wz, np.float32)
    Wf = np.asarray(Wf, np.float32)
    Wh = np.asarray(Wh, np.float32)
    wa_f = np.asarray(wa, np.float32)
    wzT = _chunkT(Wz).astype(bf)
    wfzT = _chunkT(Wf[:, :H]).astype(bf)
    whnT = _chunkT(Wh[:, :H]).astype(bf)
    whzT = _chunkT(Wh[:, H:]).astype(bf)
    wfeT = np.ascontiguousarray(Wf[:, H:].T).astype(bf)
    w2T = _chunkT(wa_f[:2 * H].reshape(2, H)).astype(bf)
    bzb = np.asarray(bz, np.float32)[None, :].astype(bf)
    waeb = wa_f[2 * H:][None, :].astype(bf)
    bab = np.asarray(ba, np.float32).reshape(1, 1)

    in_maps = []
    for c in range(NCORES):
        s = slice(c * PB, (c + 1) * PB)
        in_maps.append(dict(
            berts=bs[s], bertsT=np.ascontiguousarray(bertsT[s]),
            dep=dep[s], adjf=adjf[s],
            vrow=np.ascontiguousarray(vrow_full[s][None, :, :]),
            wzT=wzT, wfzT=wfzT, whnT=whnT, whzT=whzT, wfeT=wfeT, w2T=w2T,
            bzt=bzb, wae=waeb, bat=bab,
        ))
    return in_maps


def kernel(bert_hidden_states, dep_type_adj, deprel_adj, asp_start, asp_end,
           Wz, bz, wa, ba, Wf, Wh):
    from concourse.bass_utils import run_bass_kernel_spmd

    in_maps = _prep_in_maps(bert_hidden_states, dep_type_adj, deprel_adj,
                            asp_start, asp_end, Wz, bz, wa, ba, Wf, Wh)
    nc = _get_nc()
    res = run_bass_kernel_spmd(nc, in_maps, core_ids=list(range(NCORES)),
                               trace=bool(_CACHED.get("trace")),
                               tmpdir=_CACHED.get("trace_tmpdir"))
    _CACHED["last_results"] = res
    outs = [res.results[c]["out"] for c in range(NCORES)]
    return np.concatenate(outs, axis=0).astype(np.float32)



# revision 30
# speedup vs baseline: 1.2986x; 1.0063x over previous
"""Trainium2 Bass kernel for AspectNeighborAttention (gnn_message_passing).

Pure data-parallel over batch: 32 batches -> 8 NeuronCores x 4 batches.
All weights replicated, host-converted to bf16 and host-PRE-TRANSPOSED into
the chunk-major [128, KC, *] lhsT/rhs layouts the TensorEngine wants, so the
device does plain contiguous DMAs only. dep is host-bf16 (halves HBM traffic).

Per-core dataflow for each batch b (L=128 tokens, H=768, E=64, KC=6):
  zs^T   = Wz @ bertS^T + bz            (PE, bf16, packed PSUM groups)
  s_i,s_j= [wa_i;wa_j] @ zs^T           (PE, packed [1,128] regions)
  s_e    = reduce_e(dep * wa_e)         (DVE bf16 2x passes)
  score  = lrelu(s_i + s_j + s_e + ba)  (PE rank-1 bcast + DVE + ACT)
  attn   = mask * softmax(...)          (additive-shift masking, exp on ACT)
  D      = reduce_j(attn * dep)         (mult split Pool/DVE, bf16 2x reduce)
  nbr^T  = per-h-chunk matmuls from A/attn^T/D^T   (PE)
  temp   = nbr @ WhN^T + zs @ WhZ^T     (PE)
  out    = upd ? temp : bert            (blend, row-rolled DMA store)

The roll(z,-1)/roll(out,+1) pair is handled purely with shifted-row DMAs.
"""

import sys

for _p in ("/opt/trn_rl_repo",):
    if _p not in sys.path:
        sys.path.insert(0, _p)

import os
import numpy as np
import ml_dtypes

import concourse.bass as bass
import concourse.bacc as bacc_mod
import concourse.mybir as mybir
import concourse.tile as tile
from concourse.masks import make_identity

B, L, H, E = 32, 128, 768, 64
NCORES = 8
PB = B // NCORES  # batches per core
KC = H // 128     # 6 k-chunks
F32 = mybir.dt.float32
BF16 = mybir.dt.bfloat16
AF = mybir.ActivationFunctionType
OP = mybir.AluOpType
AX = mybir.AxisListType
MASK_SHIFT = 10000.0  # additive mask offset (see score masking)

_CACHED = {}

CFG = dict(
    dep_bufs=int(os.environ.get("K_DEP_BUFS", 2)),
    ttmp_bufs=int(os.environ.get("K_TTMP_BUFS", 3)),
    spool_bufs=int(os.environ.get("K_SPOOL_BUFS", 3)),
    opool_bufs=int(os.environ.get("K_OPOOL_BUFS", 3)),
    ptr_bufs=int(os.environ.get("K_PTR_BUFS", 3)),
    pbig_bufs=int(os.environ.get("K_PBIG_BUFS", 2)),
    jpd=int(os.environ.get("K_JPD", 36)),  # D-mult j-split: [0,jpd) Pool
    jp1=int(os.environ.get("K_JP1", 38)),  # s_e-mult j-split: [0,jp1) Pool
)


def _build(debug=False):
    nc = bacc_mod.Bacc("TRN2", target_bir_lowering=False, debug=False,
                       num_devices=NCORES)

    bert = nc.dram_tensor("berts", [PB, L, H], F32, kind="ExternalInput")
    bertsT = nc.dram_tensor("bertsT", [PB, 128, KC, 128], BF16,
                            kind="ExternalInput")
    dep = nc.dram_tensor("dep", [PB, L, L, E], BF16, kind="ExternalInput")
    adjf = nc.dram_tensor("adjf", [PB, L, L], F32, kind="ExternalInput")
    vrow = nc.dram_tensor("vrow", [1, PB, 128], F32, kind="ExternalInput")
    wzT_d = nc.dram_tensor("wzT", [128, KC, H], BF16, kind="ExternalInput")
    wfzT_d = nc.dram_tensor("wfzT", [128, KC, H], BF16, kind="ExternalInput")
    whnT_d = nc.dram_tensor("whnT", [128, KC, H], BF16, kind="ExternalInput")
    whzT_d = nc.dram_tensor("whzT", [128, KC, H], BF16, kind="ExternalInput")
    wfeT_d = nc.dram_tensor("wfeT", [E, H], BF16, kind="ExternalInput")
    w2T_d = nc.dram_tensor("w2T", [128, KC, 2], BF16, kind="ExternalInput")
    bzt = nc.dram_tensor("bzt", [1, H], BF16, kind="ExternalInput")
    wae = nc.dram_tensor("wae", [1, E], BF16, kind="ExternalInput")
    bat = nc.dram_tensor("bat", [1, 1], F32, kind="ExternalInput")
    out = nc.dram_tensor("out", [PB, L, H], F32, kind="ExternalOutput")

    dbg = {}
    if debug:
        for nm, shape, dt in [
            ("d_zsT", [128, KC, 128], BF16), ("d_si", [1, 128], F32),
            ("d_sjb", [1, 128], F32), ("d_se", [128, L], BF16),
            ("d_masked", [128, L], F32), ("d_attn", [128, L], BF16),
            ("d_dvec", [128, E], BF16), ("d_ab", [128, H], BF16),
            ("d_nbrT", [128, KC, 128], BF16), ("d_tempb", [128, H], F32),
            ("d_upd", [128, 1], F32), ("d_scb", [128, 128], F32),
        ]:
            dbg[nm] = nc.dram_tensor(nm, shape, dt, kind="ExternalOutput")
    with tile.TileContext(nc) as tc:
        with nc.allow_low_precision("bf16 softmax/D path, 2e-2 rel-err gate"):
            _body(tc, nc, bert, bertsT, dep, adjf, vrow, wzT_d, wfzT_d,
                  whnT_d, whzT_d, wfeT_d, w2T_d, bzt, wae, bat, out, dbg)
    nc.compile()
    return nc


def _body(tc, nc, bert, bertsT, dep, adjf, vrow, wzT_d, wfzT_d,
          whnT_d, whzT_d, wfeT_d, w2T_d, bzt, wae, bat, out, dbg=None):
    def dump(name, ap):
        if dbg and name in dbg:
            nc.sync.dma_start(dbg[name][...], ap)
    import contextlib
    cfg = CFG
    JPD = cfg["jpd"]
    JP1 = cfg["jp1"]
    ctx = contextlib.ExitStack()
    with ctx:
        wpool = ctx.enter_context(tc.tile_pool(name="weights", bufs=1))
        dpool = ctx.enter_context(
            tc.tile_pool(name="dep", bufs=cfg["dep_bufs"]))
        tpool = ctx.enter_context(
            tc.tile_pool(name="ttmp", bufs=cfg["ttmp_bufs"]))
        spool = ctx.enter_context(
            tc.tile_pool(name="small", bufs=cfg["spool_bufs"]))
        opool = ctx.enter_context(
            tc.tile_pool(name="outp", bufs=cfg["opool_bufs"]))
        p_tr = ctx.enter_context(
            tc.tile_pool(name="p_tr", bufs=cfg["ptr_bufs"], space="PSUM"))
        p_big = ctx.enter_context(
            tc.tile_pool(name="p_big", bufs=cfg["pbig_bufs"], space="PSUM"))

        # ---------------- one-time setup (plain DMAs only) ----------------
        wzT = wpool.tile([128, KC, H], BF16, tag="wzT")
        nc.sync.dma_start(wzT[:], wzT_d[...])
        wfzT = wpool.tile([128, KC, H], BF16, tag="wfzT")
        nc.sync.dma_start(wfzT[:], wfzT_d[...])
        whnT = wpool.tile([128, KC, H], BF16, tag="whnT")
        nc.sync.dma_start(whnT[:], whnT_d[...])
        whzT = wpool.tile([128, KC, H], BF16, tag="whzT")
        nc.sync.dma_start(whzT[:], whzT_d[...])
        wfeT = wpool.tile([E, H], BF16, tag="wfeT")
        nc.sync.dma_start(wfeT[:], wfeT_d[...])
        w2T = wpool.tile([128, KC, 2], BF16, tag="w2T")
        nc.sync.dma_start(w2T[:], w2T_d[...])
        bzr = wpool.tile([1, H], BF16, tag="bzr")
        nc.sync.dma_start(bzr[:], bzt[:, :])
        waer = wpool.tile([1, E], BF16, tag="waer")
        nc.sync.dma_start(waer[:], wae[:, :])
        bar = wpool.tile([1, 1], F32, tag="bar")
        nc.sync.dma_start(bar[:], bat[:, :])
        vrow4 = wpool.tile([1, PB, 128], F32, tag="vrow4")
        nc.sync.dma_start(vrow4[:], vrow[:, :, :])

        ones_f = wpool.tile([1, 128], F32, tag="ones_f")
        nc.gpsimd.memset(ones_f[:], 1.0)
        ones_b = wpool.tile([1, 128], BF16, tag="ones_b")
        nc.gpsimd.memset(ones_b[:], 1.0)
        id_bf = wpool.tile([128, 128], BF16, tag="id_bf")
        make_identity(nc, id_bf[:])

        # wa_e broadcast to all partitions via rank-1 matmul
        p_wae = p_tr.tile([128, 512], F32, tag="p_tr")
        nc.tensor.matmul(p_wae[:, 0:E], ones_b[:], waer[:],
                         start=True, stop=True)
        wae_bc = wpool.tile([128, E], BF16, tag="wae_bc")
        nc.scalar.copy(wae_bc[:], p_wae[:, 0:E])
        # dense [128, L, E] replica so the s_e mult runs in 2x dense mode
        wae_rep = wpool.tile([128, L, E], BF16, tag="wae_rep")
        nc.vector.tensor_copy(
            wae_rep[:], wae_bc[:].unsqueeze(1).broadcast_to([128, L, E]))

        # -------- per-batch pipeline, software-pipelined --------
        # The blend/store of batch b-1 is emitted mid-iteration-b so it
        # never head-of-line-blocks the next batch's front-end work on the
        # in-order engine queues.
        def emit_front(b):
            st = {}
            # bertS: rows shifted by one token (z roll); f32 exact for blend
            bertS = spool.tile([128, H], F32, tag="bertS")
            nc.sync.dma_start(bertS[:], bert[b, :, :])
            bertST = spool.tile([128, KC, 128], BF16, tag="bertST")
            nc.sync.dma_start(bertST[:], bertsT[b, :, :, :])
            dept = dpool.tile([128, L, E], BF16, tag="dept")
            nc.sync.dma_start(dept[:, 0:64, :], dep[b, :, 0:64, :])
            nc.sync.dma_start(dept[:, 64:L, :], dep[b, :, 64:L, :])
            adjt = spool.tile([128, L], F32, tag="adjt")
            nc.sync.dma_start(adjt[:], adjf[b, :, :])
            st.update(bertS=bertS, dept=dept, adjt=adjt)

            # ---- zs^T = Wz @ bertS^T + bz ----
            p_z = p_big.tile([128, H], F32, tag="p_big")
            for hc in range(KC):
                ns = slice(hc * 128, (hc + 1) * 128)
                for kc in range(KC):
                    nc.tensor.matmul(p_z[:, ns], wzT[:, kc, ns],
                                     bertST[:, kc, :],
                                     start=(kc == 0), stop=False)
                nc.tensor.matmul(p_z[:, ns], bzr[0:1, ns], ones_b[:],
                                 start=False, stop=True)
            zsT = spool.tile([128, KC, 128], BF16, tag="zsT")
            nc.scalar.copy(zsT[:], p_z[:])
            if b == 0:
                dump("d_zsT", zsT[:])

            # ---- s_i col, (s_j + ba) row-bcast score base ----
            p_s3 = p_tr.tile([128, 512], F32, tag="p_tr")
            for kc in range(KC):
                nc.tensor.matmul(p_s3[0:1, 0:128], w2T[:, kc, 0:1],
                                 zsT[:, kc, :],
                                 start=(kc == 0), stop=(kc == KC - 1))
            for kc in range(KC):
                nc.tensor.matmul(p_s3[0:1, 128:256], w2T[:, kc, 1:2],
                                 zsT[:, kc, :],
                                 start=(kc == 0), stop=(kc == KC - 1))
            si_row = spool.tile([1, 128], F32, tag="si_row")
            nc.scalar.copy(si_row[:], p_s3[0:1, 0:128])
            sjb = spool.tile([1, 128], F32, tag="sjb")
            nc.vector.tensor_scalar(sjb[:], p_s3[0:1, 128:256], bar[0:1, 0:1],
                                    None, op0=OP.add)
            nc.tensor.matmul(p_s3[:, 384:385], si_row[:], ones_f[0:1, 0:1],
                             start=True, stop=True)
            nc.tensor.matmul(p_s3[:, 256:384], ones_f[:], sjb[:],
                             start=True, stop=True)
            if b == 0:
                dump("d_si", si_row[:])
                dump("d_sjb", sjb[:])

            # ---- s_e = reduce_e(dep * wa_e); dense mult split Pool/DVE ----
            tmp1 = tpool.tile([128, L, E], BF16, tag="ttmp")
            nc.gpsimd.tensor_tensor(
                tmp1[:, 0:JP1, :], dept[:, 0:JP1, :],
                wae_rep[:, 0:JP1, :], op=OP.mult)
            nc.vector.tensor_tensor(
                tmp1[:, JP1:L, :], dept[:, JP1:L, :],
                wae_rep[:, JP1:L, :], op=OP.mult)
            se = spool.tile([128, L], BF16, tag="se")
            nc.vector.tensor_reduce(se[:], tmp1[:], axis=AX.X, op=OP.add)
            if b == 0:
                dump("d_se", se[:])

            # ---- score = lrelu(se + si + sj + ba); masked; softmax ----
            sadd = spool.tile([128, L], F32, tag="sadd")
            nc.vector.scalar_tensor_tensor(
                sadd[:], se[:], p_s3[:, 384:385], p_s3[:, 256:384],
                op0=OP.add, op1=OP.add)
            score = spool.tile([128, L], F32, tag="score")
            nc.vector.scalar_tensor_tensor(
                score[:], sadd[:], 0.01, sadd[:], op0=OP.mult, op1=OP.max)
            masked = spool.tile([128, L], F32, tag="masked")
            nc.vector.scalar_tensor_tensor(
                masked[:], score[:], MASK_SHIFT, adjt[:],
                op0=OP.add, op1=OP.mult)
            if b == 0:
                dump("d_masked", masked[:])
            mxn = spool.tile([128, 1], F32, tag="mxn")
            nc.vector.tensor_reduce(mxn[:], masked[:], axis=AX.X, op=OP.max,
                                    negate=True)
            ex = spool.tile([128, L], F32, tag="ex")
            sumex = spool.tile([128, 1], F32, tag="sumex")
            nc.scalar.activation(ex[:], masked[:], AF.Exp, bias=mxn[:],
                                 scale=1.0, accum_out=sumex[:])
            rec = spool.tile([128, 1], F32, tag="rec")
            nc.vector.reciprocal(rec[:], sumex[:])
            attnb = spool.tile([128, L], BF16, tag="attnb")
            nc.vector.scalar_tensor_tensor(
                attnb[:], ex[:], rec[:], adjt[:], op0=OP.mult, op1=OP.mult)
            if b == 0:
                dump("d_attn", attnb[:])

            # attn^T transpose early (needs only attnb)
            p_ad = p_tr.tile([128, 256], BF16, tag="p_tr")
            nc.tensor.transpose(p_ad[:, 0:128], attnb[:], id_bf[:])
            attnT = spool.tile([128, 128], BF16, tag="attnT")
            nc.scalar.copy(attnT[:], p_ad[:, 0:128])
            st.update(zsT=zsT, attnb=attnb, p_ad=p_ad, attnT=attnT)
            return st

        def emit_blend(b, st):
            tempb = opool.tile([128, H], F32, tag="tempb")
            nc.scalar.copy(tempb[:], st["p_t"][:])
            if b == 0:
                dump("d_tempb", tempb[:])
            tdiff = opool.tile([128, H], F32, tag="tdiff")
            nc.gpsimd.tensor_tensor(tdiff[:], tempb[:], st["bertS"][:],
                                    op=OP.subtract)
            outt = opool.tile([128, H], F32, tag="outt")
            nc.vector.scalar_tensor_tensor(
                outt[:], tdiff[:], st["p_v"][:, 0:1], st["bertS"][:],
                op0=OP.mult, op1=OP.add)
            nc.sync.dma_start(out[b, 1:128, :], outt[0:127, :])
            nc.sync.dma_start(out[b, 0:1, :], outt[127:128, :])

        def emit_back(b, st):
            zsT, dept, attnb = st["zsT"], st["dept"], st["attnb"]
            # ---- A = zs @ WfZ^T (overlaps the D reduction below) ----
            p_a = p_big.tile([128, H], F32, tag="p_big")
            for ns in (slice(0, 512), slice(512, H)):
                for kc in range(KC):
                    nc.tensor.matmul(p_a[:, ns], zsT[:, kc, :],
                                     wfzT[:, kc, ns],
                                     start=(kc == 0), stop=(kc == KC - 1))
            ab = spool.tile([128, H], BF16, tag="ab")
            nc.scalar.copy(ab[:], p_a[:])
            if b == 0:
                dump("d_ab", ab[:])

            # ---- D = reduce_j(attn * dep), natural layout, Pool/DVE ----
            tmp2 = tpool.tile([128, L, E], BF16, tag="ttmp")
            nc.gpsimd.tensor_tensor(
                tmp2[:, 0:JPD, :], dept[:, 0:JPD, :],
                attnb[:, 0:JPD].unsqueeze(2).broadcast_to([128, JPD, E]),
                op=OP.mult)
            nc.vector.tensor_tensor(
                tmp2[:, JPD:L, :], dept[:, JPD:L, :],
                attnb[:, JPD:L].unsqueeze(2).broadcast_to([128, L - JPD, E]),
                op=OP.mult)
            tmp3 = tpool.tile([128, L, E], BF16, tag="ttmp")
            src, dst = tmp2, tmp3
            h = 32
            while h >= 1:
                nc.gpsimd.tensor_tensor(
                    dst[:, 0:h, :], src[:, 0:h, :], src[:, h:2 * h, :],
                    op=OP.add)
                nc.vector.tensor_tensor(
                    dst[:, 64:64 + h, :], src[:, 64:64 + h, :],
                    src[:, 64 + h:64 + 2 * h, :], op=OP.add)
                src, dst = dst, src
                h //= 2
            dvb = spool.tile([128, E], BF16, tag="dvb")
            nc.vector.tensor_tensor(dvb[:], src[:, 0, :], src[:, 64, :],
                                    op=OP.add)
            if b == 0:
                dump("d_dvec", dvb[:])

            # D^T via PE transpose
            p_ad = st["p_ad"]
            nc.tensor.transpose(p_ad[0:E, 128:256], dvb[:], id_bf[:])
            dT = spool.tile([E, 128], BF16, tag="dT")
            nc.scalar.copy(dT[:], p_ad[0:E, 128:256])

            # ---- nbr^T per h-chunk ----
            p_n = p_big.tile([128, H], F32, tag="p_big")
            for hc in range(KC):
                ns = slice(hc * 128, (hc + 1) * 128)
                nc.tensor.matmul(p_n[:, ns], ab[:, ns], st["attnT"][:],
                                 start=True, stop=False)
                nc.tensor.matmul(p_n[:, ns], wfeT[:, ns], dT[:],
                                 start=False, stop=True)
            nbrT = spool.tile([128, KC, 128], BF16, tag="nbrT")
            nc.scalar.copy(nbrT[:], p_n[:])
            if b == 0:
                dump("d_nbrT", nbrT[:])

            # ---- temp = nbr @ WhN^T + zs @ WhZ^T (copy deferred) ----
            p_t = p_big.tile([128, H], F32, tag="p_big")
            for ns in (slice(0, 512), slice(512, H)):
                for kc in range(KC):
                    nc.tensor.matmul(p_t[:, ns], nbrT[:, kc, :],
                                     whnT[:, kc, ns],
                                     start=(kc == 0), stop=False)
                for kc in range(KC):
                    nc.tensor.matmul(p_t[:, ns], zsT[:, kc, :],
                                     whzT[:, kc, ns],
                                     start=False, stop=(kc == KC - 1))
            st["p_t"] = p_t

            # ---- upd mask column (host-folded span & any-neighbor) ----
            p_v = p_tr.tile([128, 512], F32, tag="p_tr")
            nc.tensor.matmul(p_v[:, 0:1], vrow4[0:1, b, :], ones_f[0:1, 0:1],
                             start=True, stop=True)
            st["p_v"] = p_v

        prev = None
        for b in range(PB):
            st = emit_front(b)
            if prev is not None:
                emit_blend(b - 1, prev)
            emit_back(b, st)
            prev = st
        emit_blend(PB - 1, prev)


def _get_nc():
    if "nc" not in _CACHED:
        _CACHED["nc"] = _build()
    return _CACHED["nc"]


def _chunkT(w):
    """W [rows, K] -> W^T chunk-major [128, K//128, rows] (lhsT layout)."""
    rows, k = w.shape
    return np.ascontiguousarray(
        w.T.reshape(k // 128, 128, rows).transpose(1, 0, 2))


def _prep_in_maps(bert_hidden_states, dep_type_adj, deprel_adj,
                  asp_start, asp_end, Wz, bz, wa, ba, Wf, Wh):
    bf = ml_dtypes.bfloat16
    bert = np.ascontiguousarray(np.asarray(bert_hidden_states, np.float32))
    dep = np.asarray(dep_type_adj, np.float32).astype(bf)
    adjf = np.ascontiguousarray(np.asarray(deprel_adj).astype(np.float32))
    # bertS^T chunk-major per batch: rows shifted by one (the z-roll)
    bs = np.ascontiguousarray(np.roll(bert, -1, axis=1))
    bertsT = np.ascontiguousarray(
        bs.transpose(0, 2, 1).reshape(B, KC, 128, L).transpose(0, 2, 1, 3)
    ).astype(bf)
    pos = np.arange(L, dtype=np.float32)
    s_ = np.asarray(asp_start).astype(np.float32)[:, None]
    e_ = np.asarray(asp_end).astype(np.float32)[:, None]
    vrow_full = (((pos[None, :] >= s_) & (pos[None, :] <= e_))
                 & (np.asarray(deprel_adj) > 0).any(-1)).astype(np.float32)

    Wz = np.asarray(Wz, np.float32)
    Wf = np.asarray(Wf, np.float32)
    Wh = np.asarray(Wh, np.float32)
    wa_f = np.asarray(wa, np.float32)
    wzT = _chunkT(Wz).astype(bf)
    wfzT = _chunkT(Wf[:, :H]).astype(bf)
    whnT = _chunkT(Wh[:, :H]).astype(bf)
    whzT = _chunkT(Wh[:, H:]).astype(bf)
    wfeT = np.ascontiguousarray(Wf[:, H:].T).astype(bf)
    w2T = _chunkT(wa_f[:2 * H].reshape(2, H)).astype(bf)
    bzb = np.asarray(bz, np.float32)[None, :].astype(bf)
    waeb = wa_f[2 * H:][None, :].astype(bf)
    bab = np.asarray(ba, np.float32).reshape(1, 1)

    in_maps = []
    for c in range(NCORES):
        s = slice(c * PB, (c + 1) * PB)
        in_maps.append(dict(
            berts=bs[s], bertsT=np.ascontiguousarray(bertsT[s]),
            dep=dep[s], adjf=adjf[s],
            vrow=np.ascontiguousarray(vrow_full[s][None, :, :]),
            wzT=wzT, wfzT=wfzT, whnT=whnT, whzT=whzT, wfeT=wfeT, w2T=w2T,
            bzt=bzb, wae=waeb, bat=bab,
        ))
    return in_maps


def kernel(bert_hidden_states, dep_type_adj, deprel_adj, asp_start, asp_end,
           Wz, bz, wa, ba, Wf, Wh):
    from concourse.bass_utils import run_bass_kernel_spmd

    in_maps = _prep_in_maps(bert_hidden_states, dep_type_adj, deprel_adj,
                            asp_start, asp_end, Wz, bz, wa, ba, Wf, Wh)
    nc = _get_nc()
    res = run_bass_kernel_spmd(nc, in_maps, core_ids=list(range(NCORES)),
                               trace=bool(_CACHED.get("trace")),
                               tmpdir=_CACHED.get("trace_tmpdir"))
    _CACHED["last_results"] = res
    outs = [res.results[c]["out"] for c in range(NCORES)]
    return np.concatenate(outs, axis=0).astype(np.float32)



# revision 31
# speedup vs baseline: 1.4078x; 1.0841x over previous
"""Trainium2 Bass kernel for AspectNeighborAttention (gnn_message_passing).

Pure data-parallel over batch: 32 batches -> 8 NeuronCores x 4 batches.
All weights replicated, host-converted to bf16 and host-PRE-TRANSPOSED into
the chunk-major [128, KC, *] lhsT/rhs layouts the TensorEngine wants, so the
device does plain contiguous DMAs only. dep is host-bf16 (halves HBM traffic).

Per-core dataflow for each batch b (L=128 tokens, H=768, E=64, KC=6):
  zs^T   = Wz @ bertS^T + bz            (PE, bf16, packed PSUM groups)
  s_i,s_j= [wa_i;wa_j] @ zs^T           (PE, packed [1,128] regions)
  s_e    = reduce_e(dep * wa_e)         (DVE bf16 2x passes)
  score  = lrelu(s_i + s_j + s_e + ba)  (PE rank-1 bcast + DVE + ACT)
  attn   = mask * softmax(...)          (additive-shift masking, exp on ACT)
  D      = reduce_j(attn * dep)         (mult split Pool/DVE, bf16 2x reduce)
  nbr^T  = per-h-chunk matmuls from A/attn^T/D^T   (PE)
  temp   = nbr @ WhN^T + zs @ WhZ^T     (PE)
  out    = upd ? temp : bert            (blend, row-rolled DMA store)

The roll(z,-1)/roll(out,+1) pair is handled purely with shifted-row DMAs.
"""

import sys

for _p in ("/opt/trn_rl_repo",):
    if _p not in sys.path:
        sys.path.insert(0, _p)

import os
import numpy as np
import ml_dtypes

import concourse.bass as bass
import concourse.bacc as bacc_mod
import concourse.mybir as mybir
import concourse.tile as tile
from concourse.masks import make_identity

B, L, H, E = 32, 128, 768, 64
NCORES = 8
PB = B // NCORES  # batches per core
KC = H // 128     # 6 k-chunks
F32 = mybir.dt.float32
BF16 = mybir.dt.bfloat16
AF = mybir.ActivationFunctionType
OP = mybir.AluOpType
AX = mybir.AxisListType
MASK_SHIFT = 10000.0  # additive mask offset (see score masking)

_CACHED = {}

CFG = dict(
    dep_bufs=int(os.environ.get("K_DEP_BUFS", 2)),
    ttmp_bufs=int(os.environ.get("K_TTMP_BUFS", 3)),
    spool_bufs=int(os.environ.get("K_SPOOL_BUFS", 3)),
    opool_bufs=int(os.environ.get("K_OPOOL_BUFS", 3)),
    ptr_bufs=int(os.environ.get("K_PTR_BUFS", 3)),
    pbig_bufs=int(os.environ.get("K_PBIG_BUFS", 2)),
    jpd=int(os.environ.get("K_JPD", 48)),  # D-mult j-split: [0,jpd) Pool
    jp1=int(os.environ.get("K_JP1", 38)),  # s_e-mult j-split: [0,jp1) Pool
)


def _build(debug=False):
    nc = bacc_mod.Bacc("TRN2", target_bir_lowering=False, debug=False,
                       num_devices=NCORES)

    bert = nc.dram_tensor("berts", [PB, L, H], F32, kind="ExternalInput")
    bertsT = nc.dram_tensor("bertsT", [PB, 128, KC, 128], BF16,
                            kind="ExternalInput")
    dep = nc.dram_tensor("dep", [PB, L, L, E], BF16, kind="ExternalInput")
    adjf = nc.dram_tensor("adjf", [PB, L, L], F32, kind="ExternalInput")
    vrow = nc.dram_tensor("vrow", [1, PB, 128], F32, kind="ExternalInput")
    wzT_d = nc.dram_tensor("wzT", [128, KC, H], BF16, kind="ExternalInput")
    g0T_d = nc.dram_tensor("g0T", [128, KC, H], BF16, kind="ExternalInput")
    whzT_d = nc.dram_tensor("whzT", [128, KC, H], BF16, kind="ExternalInput")
    g1_d = nc.dram_tensor("g1", [E, H], BF16, kind="ExternalInput")
    w2T_d = nc.dram_tensor("w2T", [128, KC, 2], BF16, kind="ExternalInput")
    bzt = nc.dram_tensor("bzt", [1, H], BF16, kind="ExternalInput")
    wae = nc.dram_tensor("wae", [1, E], BF16, kind="ExternalInput")
    bat = nc.dram_tensor("bat", [1, 1], F32, kind="ExternalInput")
    out = nc.dram_tensor("out", [PB, L, H], F32, kind="ExternalOutput")

    dbg = {}
    if debug:
        for nm, shape, dt in [
            ("d_zsT", [128, KC, 128], BF16), ("d_si", [1, 128], F32),
            ("d_sjb", [1, 128], F32), ("d_se", [128, L], BF16),
            ("d_masked", [128, L], F32), ("d_attn", [128, L], BF16),
            ("d_dvec", [128, E], BF16), ("d_ab", [128, H], BF16),
            ("d_nbrT", [128, KC, 128], BF16), ("d_tempb", [128, H], F32),
            ("d_upd", [128, 1], F32), ("d_scb", [128, 128], F32),
        ]:
            dbg[nm] = nc.dram_tensor(nm, shape, dt, kind="ExternalOutput")
    with tile.TileContext(nc) as tc:
        with nc.allow_low_precision("bf16 softmax/D path, 2e-2 rel-err gate"):
            _body(tc, nc, bert, bertsT, dep, adjf, vrow, wzT_d, g0T_d,
                  whzT_d, g1_d, w2T_d, bzt, wae, bat, out, dbg)
    nc.compile()
    return nc


def _body(tc, nc, bert, bertsT, dep, adjf, vrow, wzT_d, g0T_d,
          whzT_d, g1_d, w2T_d, bzt, wae, bat, out, dbg=None):
    def dump(name, ap):
        if dbg and name in dbg:
            nc.sync.dma_start(dbg[name][...], ap)
    import contextlib
    cfg = CFG
    JPD = cfg["jpd"]
    JP1 = cfg["jp1"]
    ctx = contextlib.ExitStack()
    with ctx:
        wpool = ctx.enter_context(tc.tile_pool(name="weights", bufs=1))
        dpool = ctx.enter_context(
            tc.tile_pool(name="dep", bufs=cfg["dep_bufs"]))
        tpool = ctx.enter_context(
            tc.tile_pool(name="ttmp", bufs=cfg["ttmp_bufs"]))
        spool = ctx.enter_context(
            tc.tile_pool(name="small", bufs=cfg["spool_bufs"]))
        opool = ctx.enter_context(
            tc.tile_pool(name="outp", bufs=cfg["opool_bufs"]))
        apool = ctx.enter_context(tc.tile_pool(name="arep", bufs=2))
        p_tr = ctx.enter_context(
            tc.tile_pool(name="p_tr", bufs=cfg["ptr_bufs"], space="PSUM"))
        p_big = ctx.enter_context(
            tc.tile_pool(name="p_big", bufs=cfg["pbig_bufs"], space="PSUM"))

        # ---------------- one-time setup (plain DMAs only) ----------------
        wzT = wpool.tile([128, KC, H], BF16, tag="wzT")
        nc.sync.dma_start(wzT[:], wzT_d[...])
        g0T = wpool.tile([128, KC, H], BF16, tag="g0T")
        nc.sync.dma_start(g0T[:], g0T_d[...])
        whzT = wpool.tile([128, KC, H], BF16, tag="whzT")
        nc.sync.dma_start(whzT[:], whzT_d[...])
        g1 = wpool.tile([E, H], BF16, tag="g1")
        nc.sync.dma_start(g1[:], g1_d[...])
        w2T = wpool.tile([128, KC, 2], BF16, tag="w2T")
        nc.sync.dma_start(w2T[:], w2T_d[...])
        bzr = wpool.tile([1, H], BF16, tag="bzr")
        nc.sync.dma_start(bzr[:], bzt[:, :])
        waer = wpool.tile([1, E], BF16, tag="waer")
        nc.sync.dma_start(waer[:], wae[:, :])
        bar = wpool.tile([1, 1], F32, tag="bar")
        nc.sync.dma_start(bar[:], bat[:, :])
        vrow4 = wpool.tile([1, PB, 128], F32, tag="vrow4")
        nc.sync.dma_start(vrow4[:], vrow[:, :, :])

        ones_f = wpool.tile([1, 128], F32, tag="ones_f")
        nc.gpsimd.memset(ones_f[:], 1.0)
        ones_b = wpool.tile([1, 128], BF16, tag="ones_b")
        nc.gpsimd.memset(ones_b[:], 1.0)
        id_bf = wpool.tile([128, 128], BF16, tag="id_bf")
        make_identity(nc, id_bf[:])

        # wa_e broadcast to all partitions via rank-1 matmul
        p_wae = p_tr.tile([128, 512], F32, tag="p_tr")
        nc.tensor.matmul(p_wae[:, 0:E], ones_b[:], waer[:],
                         start=True, stop=True)
        wae_bc = wpool.tile([128, E], BF16, tag="wae_bc")
        nc.scalar.copy(wae_bc[:], p_wae[:, 0:E])
        # dense [128, L, E] replica so the s_e mult runs in 2x dense mode
        wae_rep = wpool.tile([128, L, E], BF16, tag="wae_rep")
        nc.vector.tensor_copy(
            wae_rep[:], wae_bc[:].unsqueeze(1).broadcast_to([128, L, E]))

        # -------- per-batch pipeline, software-pipelined --------
        # The blend/store of batch b-1 is emitted mid-iteration-b so it
        # never head-of-line-blocks the next batch's front-end work on the
        # in-order engine queues.
        def emit_front(b):
            st = {}
            # bertS: rows shifted by one token (z roll); f32 exact for blend
            bertS = spool.tile([128, H], F32, tag="bertS")
            nc.sync.dma_start(bertS[:], bert[b, :, :])
            bertST = spool.tile([128, KC, 128], BF16, tag="bertST")
            nc.sync.dma_start(bertST[:], bertsT[b, :, :, :])
            dept = dpool.tile([128, L, E], BF16, tag="dept")
            nc.sync.dma_start(dept[:, 0:64, :], dep[b, :, 0:64, :])
            nc.sync.dma_start(dept[:, 64:L, :], dep[b, :, 64:L, :])
            adjt = spool.tile([128, L], F32, tag="adjt")
            nc.sync.dma_start(adjt[:], adjf[b, :, :])
            st.update(bertS=bertS, dept=dept, adjt=adjt)

            # ---- zs^T = Wz @ bertS^T + bz ----
            p_z = p_big.tile([128, H], F32, tag="p_big")
            for hc in range(KC):
                ns = slice(hc * 128, (hc + 1) * 128)
                for kc in range(KC):
                    nc.tensor.matmul(p_z[:, ns], wzT[:, kc, ns],
                                     bertST[:, kc, :],
                                     start=(kc == 0), stop=False)
                nc.tensor.matmul(p_z[:, ns], bzr[0:1, ns], ones_b[:],
                                 start=False, stop=True)
            zsT = spool.tile([128, KC, 128], BF16, tag="zsT")
            nc.scalar.copy(zsT[:], p_z[:])
            if b == 0:
                dump("d_zsT", zsT[:])

            # ---- s_i col, (s_j + ba) row-bcast score base ----
            p_s3 = p_tr.tile([128, 512], F32, tag="p_tr")
            for kc in range(KC):
                nc.tensor.matmul(p_s3[0:1, 0:128], w2T[:, kc, 0:1],
                                 zsT[:, kc, :],
                                 start=(kc == 0), stop=(kc == KC - 1))
            for kc in range(KC):
                nc.tensor.matmul(p_s3[0:1, 128:256], w2T[:, kc, 1:2],
                                 zsT[:, kc, :],
                                 start=(kc == 0), stop=(kc == KC - 1))
            si_row = spool.tile([1, 128], F32, tag="si_row")
            nc.scalar.copy(si_row[:], p_s3[0:1, 0:128])
            sjb = spool.tile([1, 128], F32, tag="sjb")
            nc.vector.tensor_scalar(sjb[:], p_s3[0:1, 128:256], bar[0:1, 0:1],
                                    None, op0=OP.add)
            nc.tensor.matmul(p_s3[:, 384:385], si_row[:], ones_f[0:1, 0:1],
                             start=True, stop=True)
            nc.tensor.matmul(p_s3[:, 256:384], ones_f[:], sjb[:],
                             start=True, stop=True)
            if b == 0:
                dump("d_si", si_row[:])
                dump("d_sjb", sjb[:])

            # ---- s_e = reduce_e(dep * wa_e); dense mult split Pool/DVE ----
            tmp1 = tpool.tile([128, L, E], BF16, tag="ttmp")
            nc.gpsimd.tensor_tensor(
                tmp1[:, 0:JP1, :], dept[:, 0:JP1, :],
                wae_rep[:, 0:JP1, :], op=OP.mult)
            nc.vector.tensor_tensor(
                tmp1[:, JP1:L, :], dept[:, JP1:L, :],
                wae_rep[:, JP1:L, :], op=OP.mult)
            se = spool.tile([128, L], BF16, tag="se")
            nc.vector.tensor_reduce(se[:], tmp1[:], axis=AX.X, op=OP.add)
            if b == 0:
                dump("d_se", se[:])

            # ---- score = lrelu(se + si + sj + ba); masked; softmax ----
            sadd = spool.tile([128, L], F32, tag="sadd")
            nc.vector.scalar_tensor_tensor(
                sadd[:], se[:], p_s3[:, 384:385], p_s3[:, 256:384],
                op0=OP.add, op1=OP.add)
            score = spool.tile([128, L], F32, tag="score")
            nc.vector.scalar_tensor_tensor(
                score[:], sadd[:], 0.01, sadd[:], op0=OP.mult, op1=OP.max)
            masked = spool.tile([128, L], F32, tag="masked")
            nc.vector.scalar_tensor_tensor(
                masked[:], score[:], MASK_SHIFT, adjt[:],
                op0=OP.add, op1=OP.mult)
            if b == 0:
                dump("d_masked", masked[:])
            mxn = spool.tile([128, 1], F32, tag="mxn")
            nc.vector.tensor_reduce(mxn[:], masked[:], axis=AX.X, op=OP.max,
                                    negate=True)
            ex = spool.tile([128, L], F32, tag="ex")
            sumex = spool.tile([128, 1], F32, tag="sumex")
            nc.scalar.activation(ex[:], masked[:], AF.Exp, bias=mxn[:],
                                 scale=1.0, accum_out=sumex[:])
            rec = spool.tile([128, 1], F32, tag="rec")
            nc.vector.reciprocal(rec[:], sumex[:])
            attnb = spool.tile([128, L], BF16, tag="attnb")
            nc.vector.tensor_scalar(attnb[:], ex[:], rec[0:128, 0:1], None,
                                    op0=OP.mult)
            if b == 0:
                dump("d_attn", attnb[:])

            # attn^T transpose early (needs only attnb)
            p_ad = p_tr.tile([128, 256], BF16, tag="p_tr")
            nc.tensor.transpose(p_ad[:, 0:128], attnb[:], id_bf[:])
            attnT = spool.tile([128, 128], BF16, tag="attnT")
            nc.scalar.copy(attnT[:], p_ad[:, 0:128])
            st.update(zsT=zsT, attnb=attnb, p_ad=p_ad, attnT=attnT)
            return st

        def emit_blend(b, st):
            tempb = opool.tile([128, H], F32, tag="tempb")
            nc.scalar.copy(tempb[:], st["p_t"][:])
            if b == 0:
                dump("d_tempb", tempb[:])
            tdiff = opool.tile([128, H], F32, tag="tdiff")
            nc.gpsimd.tensor_tensor(tdiff[:], tempb[:], st["bertS"][:],
                                    op=OP.subtract)
            outt = opool.tile([128, H], F32, tag="outt")
            nc.vector.scalar_tensor_tensor(
                outt[:], tdiff[:], st["p_v"][:, 0:1], st["bertS"][:],
                op0=OP.mult, op1=OP.add)
            nc.sync.dma_start(out[b, 1:128, :], outt[0:127, :])
            nc.sync.dma_start(out[b, 0:1, :], outt[127:128, :])

        def emit_back(b, st):
            zsT, dept, attnb = st["zsT"], st["dept"], st["attnb"]
            # ---- A2 = zs @ G0^T  (G0 = WhN @ WfZ host-folded) ----
            p_a = p_big.tile([128, H], F32, tag="p_big")
            for ns in (slice(0, 512), slice(512, H)):
                for kc in range(KC):
                    nc.tensor.matmul(p_a[:, ns], zsT[:, kc, :],
                                     g0T[:, kc, ns],
                                     start=(kc == 0), stop=(kc == KC - 1))
            a2b = spool.tile([128, H], BF16, tag="ab")
            nc.scalar.copy(a2b[:], p_a[:])

            # ---- temp partial: zs@WhZ^T + attn@A2 accumulate early ----
            p_t = p_big.tile([128, H], F32, tag="p_big")
            for ns in (slice(0, 512), slice(512, H)):
                for kc in range(KC):
                    nc.tensor.matmul(p_t[:, ns], zsT[:, kc, :],
                                     whzT[:, kc, ns],
                                     start=(kc == 0), stop=False)
                nc.tensor.matmul(p_t[:, ns], st["attnT"][:], a2b[:, ns],
                                 start=False, stop=False)

            # ---- D = reduce_j(attn * dep); Pool share densified via ACT ----
            arep = apool.tile([128, JPD, E], BF16, tag="arep")
            nc.scalar.copy(
                arep[:], attnb[:, 0:JPD].unsqueeze(2).broadcast_to(
                    [128, JPD, E]))
            tmp2 = tpool.tile([128, L, E], BF16, tag="ttmp")
            nc.gpsimd.tensor_tensor(
                tmp2[:, 0:JPD, :], dept[:, 0:JPD, :], arep[:], op=OP.mult)
            nc.vector.tensor_tensor(
                tmp2[:, JPD:L, :], dept[:, JPD:L, :],
                attnb[:, JPD:L].unsqueeze(2).broadcast_to([128, L - JPD, E]),
                op=OP.mult)
            # asymmetric pairwise tree: DVE folds j[0:96), Pool j[96:128)
            tmp3 = tpool.tile([128, L, E], BF16, tag="ttmp")
            t2, t3 = tmp2, tmp3
            nc.vector.tensor_tensor(t3[:, 0:48, :], t2[:, 0:48, :],
                                    t2[:, 48:96, :], op=OP.add)
            nc.gpsimd.tensor_tensor(t3[:, 96:112, :], t2[:, 96:112, :],
                                    t2[:, 112:128, :], op=OP.add)
            nc.vector.tensor_tensor(t2[:, 0:24, :], t3[:, 0:24, :],
                                    t3[:, 24:48, :], op=OP.add)
            nc.gpsimd.tensor_tensor(t2[:, 96:104, :], t3[:, 96:104, :],
                                    t3[:, 104:112, :], op=OP.add)
            nc.vector.tensor_tensor(t3[:, 0:12, :], t2[:, 0:12, :],
                                    t2[:, 12:24, :], op=OP.add)
            nc.gpsimd.tensor_tensor(t3[:, 96:100, :], t2[:, 96:100, :],
                                    t2[:, 100:104, :], op=OP.add)
            nc.vector.tensor_tensor(t2[:, 0:6, :], t3[:, 0:6, :],
                                    t3[:, 6:12, :], op=OP.add)
            nc.gpsimd.tensor_tensor(t2[:, 96:98, :], t3[:, 96:98, :],
                                    t3[:, 98:100, :], op=OP.add)
            nc.vector.tensor_tensor(t3[:, 0:3, :], t2[:, 0:3, :],
                                    t2[:, 3:6, :], op=OP.add)
            nc.gpsimd.tensor_tensor(t3[:, 96:97, :], t2[:, 96:97, :],
                                    t2[:, 97:98, :], op=OP.add)
            nc.vector.tensor_tensor(t2[:, 0:1, :], t3[:, 0:1, :],
                                    t3[:, 1:2, :], op=OP.add)
            nc.gpsimd.tensor_tensor(t2[:, 96:97, :], t3[:, 2:3, :],
                                    t3[:, 96:97, :], op=OP.add)
            dvb = spool.tile([128, E], BF16, tag="dvb")
            nc.vector.tensor_tensor(dvb[:], t2[:, 0, :], t2[:, 96, :],
                                    op=OP.add)
            if b == 0:
                dump("d_dvec", dvb[:])

            # D^T via PE transpose
            p_ad = st["p_ad"]
            nc.tensor.transpose(p_ad[0:E, 128:256], dvb[:], id_bf[:])
            dT = spool.tile([E, 128], BF16, tag="dT")
            nc.scalar.copy(dT[:], p_ad[0:E, 128:256])

            # ---- temp final: += D @ G1 (G1 = (WhN @ WfE)^T host-folded) ----
            for ns in (slice(0, 512), slice(512, H)):
                nc.tensor.matmul(p_t[:, ns], dT[:], g1[:, ns],
                                 start=False, stop=True)
            st["p_t"] = p_t

            # ---- upd mask column (host-folded span & any-neighbor) ----
            p_v = p_tr.tile([128, 512], F32, tag="p_tr")
            nc.tensor.matmul(p_v[:, 0:1], vrow4[0:1, b, :], ones_f[0:1, 0:1],
                             start=True, stop=True)
            st["p_v"] = p_v

        prev = None
        for b in range(PB):
            st = emit_front(b)
            emit_back(b, st)
            if prev is not None:
                emit_blend(b - 1, prev)
            prev = st
        emit_blend(PB - 1, prev)


def _get_nc():
    if "nc" not in _CACHED:
        _CACHED["nc"] = _build()
    return _CACHED["nc"]


def _chunkT(w):
    """W [rows, K] -> W^T chunk-major [128, K//128, rows] (lhsT layout)."""
    rows, k = w.shape
    return np.ascontiguousarray(
        w.T.reshape(k // 128, 128, rows).transpose(1, 0, 2))


def _prep_in_maps(bert_hidden_states, dep_type_adj, deprel_adj,
                  asp_start, asp_end, Wz, bz, wa, ba, Wf, Wh):
    bf = ml_dtypes.bfloat16
    bert = np.ascontiguousarray(np.asarray(bert_hidden_states, np.float32))
    dep = np.asarray(dep_type_adj, np.float32).astype(bf)
    adjf = np.ascontiguousarray(np.asarray(deprel_adj).astype(np.float32))
    # bertS^T chunk-major per batch: rows shifted by one (the z-roll)
    bs = np.ascontiguousarray(np.roll(bert, -1, axis=1))
    bertsT = np.ascontiguousarray(
        bs.transpose(0, 2, 1).reshape(B, KC, 128, L).transpose(0, 2, 1, 3)
    ).astype(bf)
    pos = np.arange(L, dtype=np.float32)
    s_ = np.asarray(asp_start).astype(np.float32)[:, None]
    e_ = np.asarray(asp_end).astype(np.float32)[:, None]
    vrow_full = (((pos[None, :] >= s_) & (pos[None, :] <= e_))
                 & (np.asarray(deprel_adj) > 0).any(-1)).astype(np.float32)

    Wz = np.asarray(Wz, np.float32)
    Wf = np.asarray(Wf, np.float32)
    Wh = np.asarray(Wh, np.float32)
    wa_f = np.asarray(wa, np.float32)
    wzT = _chunkT(Wz).astype(bf)
    g0T = _chunkT(Wh[:, :H] @ Wf[:, :H]).astype(bf)
    whzT = _chunkT(Wh[:, H:]).astype(bf)
    g1 = np.ascontiguousarray((Wh[:, :H] @ Wf[:, H:]).T).astype(bf)
    w2T = _chunkT(wa_f[:2 * H].reshape(2, H)).astype(bf)
    bzb = np.asarray(bz, np.float32)[None, :].astype(bf)
    waeb = wa_f[2 * H:][None, :].astype(bf)
    bab = np.asarray(ba, np.float32).reshape(1, 1)

    in_maps = []
    for c in range(NCORES):
        s = slice(c * PB, (c + 1) * PB)
        in_maps.append(dict(
            berts=bs[s], bertsT=np.ascontiguousarray(bertsT[s]),
            dep=dep[s], adjf=adjf[s],
            vrow=np.ascontiguousarray(vrow_full[s][None, :, :]),
            wzT=wzT, g0T=g0T, whzT=whzT, g1=g1, w2T=w2T,
            bzt=bzb, wae=waeb, bat=bab,
        ))
    return in_maps


def kernel(bert_hidden_states, dep_type_adj, deprel_adj, asp_start, asp_end,
           Wz, bz, wa, ba, Wf, Wh):
    from concourse.bass_utils import run_bass_kernel_spmd

    in_maps = _prep_in_maps(bert_hidden_states, dep_type_adj, deprel_adj,
                            asp_start, asp_end, Wz, bz, wa, ba, Wf, Wh)
    nc = _get_nc()
    res = run_bass_kernel_spmd(nc, in_maps, core_ids=list(range(NCORES)),
                               trace=bool(_CACHED.get("trace")),
                               tmpdir=_CACHED.get("trace_tmpdir"))
    _CACHED["last_results"] = res
    outs = [res.results[c]["out"] for c in range(NCORES)]
    return np.concatenate(outs, axis=0).astype(np.float32)



# revision 33
# speedup vs baseline: 1.7745x; 1.2604x over previous
"""Trainium2 Bass kernel for AspectNeighborAttention (gnn_message_passing).

Pure data-parallel over batch: 32 batches -> 8 NeuronCores x 4 batches.
All weights replicated, host-converted to bf16 and host-PRE-TRANSPOSED into
the chunk-major [128, KC, *] lhsT/rhs layouts the TensorEngine wants, so the
device does plain contiguous DMAs only. dep is host-bf16 (halves HBM traffic).

Per-core dataflow for each batch b (L=128 tokens, H=768, E=64, KC=6):
  zs^T   = Wz @ bertS^T + bz            (PE, bf16, packed PSUM groups)
  s_i,s_j= [wa_i;wa_j] @ zs^T           (PE, packed [1,128] regions)
  s_e    = reduce_e(dep * wa_e)         (DVE bf16 2x passes)
  score  = lrelu(s_i + s_j + s_e + ba)  (PE rank-1 bcast + DVE + ACT)
  attn   = mask * softmax(...)          (additive-shift masking, exp on ACT)
  D      = reduce_j(attn * dep)         (mult split Pool/DVE, bf16 2x reduce)
  nbr^T  = per-h-chunk matmuls from A/attn^T/D^T   (PE)
  temp   = nbr @ WhN^T + zs @ WhZ^T     (PE)
  out    = upd ? temp : bert            (blend, row-rolled DMA store)

The roll(z,-1)/roll(out,+1) pair is handled purely with shifted-row DMAs.
"""

import sys

for _p in ("/opt/trn_rl_repo",):
    if _p not in sys.path:
        sys.path.insert(0, _p)

import os
import numpy as np
import ml_dtypes

import concourse.bass as bass
import concourse.bacc as bacc_mod
import concourse.mybir as mybir
import concourse.tile as tile
from concourse.masks import make_identity

B, L, H, E = 32, 128, 768, 64
NCORES = 8
PB = B // NCORES  # batches per core
KC = H // 128     # 6 k-chunks
F32 = mybir.dt.float32
BF16 = mybir.dt.bfloat16
AF = mybir.ActivationFunctionType
OP = mybir.AluOpType
AX = mybir.AxisListType
MASK_SHIFT = 10000.0  # additive mask offset (see score masking)

_CACHED = {}

CFG = dict(
    dep_bufs=int(os.environ.get("K_DEP_BUFS", 2)),
    ttmp_bufs=int(os.environ.get("K_TTMP_BUFS", 3)),
    spool_bufs=int(os.environ.get("K_SPOOL_BUFS", 3)),
    opool_bufs=int(os.environ.get("K_OPOOL_BUFS", 3)),
    ptr_bufs=int(os.environ.get("K_PTR_BUFS", 3)),
    pbig_bufs=int(os.environ.get("K_PBIG_BUFS", 2)),
    jpd=int(os.environ.get("K_JPD", 32)),  # D-mult j-split: [0,jpd) Pool
    jp1=int(os.environ.get("K_JP1", 32)),  # s_e-mult j-split: [0,jp1) Pool
)


def _build(debug=False):
    nc = bacc_mod.Bacc("TRN2", target_bir_lowering=False, debug=False,
                       num_devices=NCORES)

    bert = nc.dram_tensor("berts", [PB, L, H], F32, kind="ExternalInput")
    bertsT = nc.dram_tensor("bertsT", [PB, 128, KC, 128], BF16,
                            kind="ExternalInput")
    dep = nc.dram_tensor("dep", [PB, L, L, E], BF16, kind="ExternalInput")
    adjf = nc.dram_tensor("adjf", [PB, L, L], F32, kind="ExternalInput")
    vrow = nc.dram_tensor("vrow", [1, PB, 128], F32, kind="ExternalInput")
    wzT_d = nc.dram_tensor("wzT", [128, KC, H], BF16, kind="ExternalInput")
    g0T_d = nc.dram_tensor("g0T", [128, KC, H], BF16, kind="ExternalInput")
    whzT_d = nc.dram_tensor("whzT", [128, KC, H], BF16, kind="ExternalInput")
    g1_d = nc.dram_tensor("g1", [E, H], BF16, kind="ExternalInput")
    w2T_d = nc.dram_tensor("w2T", [128, KC, 2], BF16, kind="ExternalInput")
    bzt = nc.dram_tensor("bzt", [1, H], BF16, kind="ExternalInput")
    wae = nc.dram_tensor("wae", [1, E], BF16, kind="ExternalInput")
    bat = nc.dram_tensor("bat", [1, 1], F32, kind="ExternalInput")
    out = nc.dram_tensor("out", [PB, L, H], F32, kind="ExternalOutput")

    dbg = {}
    if debug:
        for nm, shape, dt in [
            ("d_zsT", [128, KC, 128], BF16), ("d_si", [1, 128], F32),
            ("d_sjb", [1, 128], F32), ("d_se", [128, L], BF16),
            ("d_masked", [128, L], F32), ("d_attn", [128, L], BF16),
            ("d_dvec", [128, E], BF16), ("d_ab", [128, H], BF16),
            ("d_nbrT", [128, KC, 128], BF16), ("d_tempb", [128, H], F32),
            ("d_upd", [128, 1], F32), ("d_scb", [128, 128], F32),
        ]:
            dbg[nm] = nc.dram_tensor(nm, shape, dt, kind="ExternalOutput")
    with tile.TileContext(nc) as tc:
        with nc.allow_low_precision("bf16 softmax/D path, 2e-2 rel-err gate"):
            _body(tc, nc, bert, bertsT, dep, adjf, vrow, wzT_d, g0T_d,
                  whzT_d, g1_d, w2T_d, bzt, wae, bat, out, dbg)
    nc.compile()
    return nc


def _body(tc, nc, bert, bertsT, dep, adjf, vrow, wzT_d, g0T_d,
          whzT_d, g1_d, w2T_d, bzt, wae, bat, out, dbg=None):
    def dump(name, ap):
        if dbg and name in dbg:
            nc.sync.dma_start(dbg[name][...], ap)
    import contextlib
    cfg = CFG
    JPD = cfg["jpd"]
    JP1 = cfg["jp1"]
    ctx = contextlib.ExitStack()
    with ctx:
        wpool = ctx.enter_context(tc.tile_pool(name="weights", bufs=1))
        dpool = ctx.enter_context(
            tc.tile_pool(name="dep", bufs=cfg["dep_bufs"]))
        tpool = ctx.enter_context(
            tc.tile_pool(name="ttmp", bufs=cfg["ttmp_bufs"]))
        spool = ctx.enter_context(
            tc.tile_pool(name="small", bufs=cfg["spool_bufs"]))
        opool = ctx.enter_context(
            tc.tile_pool(name="outp", bufs=cfg["opool_bufs"]))
        apool = ctx.enter_context(tc.tile_pool(name="arep", bufs=2))
        p_tr = ctx.enter_context(
            tc.tile_pool(name="p_tr", bufs=cfg["ptr_bufs"], space="PSUM"))
        p_big = ctx.enter_context(
            tc.tile_pool(name="p_big", bufs=cfg["pbig_bufs"], space="PSUM"))

        # ---------------- one-time setup (plain DMAs only) ----------------
        wzT = wpool.tile([128, KC, H], BF16, tag="wzT")
        nc.sync.dma_start(wzT[:], wzT_d[...])
        g0T = wpool.tile([128, KC, H], BF16, tag="g0T")
        nc.sync.dma_start(g0T[:], g0T_d[...])
        whzT = wpool.tile([128, KC, H], BF16, tag="whzT")
        nc.sync.dma_start(whzT[:], whzT_d[...])
        g1 = wpool.tile([E, H], BF16, tag="g1")
        nc.sync.dma_start(g1[:], g1_d[...])
        w2T = wpool.tile([128, KC, 2], BF16, tag="w2T")
        nc.sync.dma_start(w2T[:], w2T_d[...])
        bzr = wpool.tile([1, H], BF16, tag="bzr")
        nc.sync.dma_start(bzr[:], bzt[:, :])
        waer = wpool.tile([1, E], BF16, tag="waer")
        nc.sync.dma_start(waer[:], wae[:, :])
        bar = wpool.tile([1, 1], F32, tag="bar")
        nc.sync.dma_start(bar[:], bat[:, :])
        vrow4 = wpool.tile([1, PB, 128], F32, tag="vrow4")
        nc.sync.dma_start(vrow4[:], vrow[:, :, :])

        ones_f = wpool.tile([1, 128], F32, tag="ones_f")
        nc.gpsimd.memset(ones_f[:], 1.0)
        ones_b = wpool.tile([1, 128], BF16, tag="ones_b")
        nc.gpsimd.memset(ones_b[:], 1.0)
        id_bf = wpool.tile([128, 128], BF16, tag="id_bf")
        make_identity(nc, id_bf[:])
        id_negb = wpool.tile([128, 128], BF16, tag="id_negb")
        nc.vector.tensor_scalar(id_negb[:], id_bf[:], -1.0, None, op0=OP.mult)

        # wa_e broadcast to all partitions via rank-1 matmul
        p_wae = p_tr.tile([128, 512], F32, tag="p_tr")
        nc.tensor.matmul(p_wae[:, 0:E], ones_b[:], waer[:],
                         start=True, stop=True)
        wae_bc = wpool.tile([128, E], BF16, tag="wae_bc")
        nc.scalar.copy(wae_bc[:], p_wae[:, 0:E])
        # dense [128, L, E] replica so the s_e mult runs in 2x dense mode
        wae_rep = wpool.tile([128, L, E], BF16, tag="wae_rep")
        nc.vector.tensor_copy(
            wae_rep[:], wae_bc[:].unsqueeze(1).broadcast_to([128, L, E]))

        # -------- per-batch pipeline, software-pipelined --------
        # The blend/store of batch b-1 is emitted mid-iteration-b so it
        # never head-of-line-blocks the next batch's front-end work on the
        # in-order engine queues.
        def emit_front(b):
            st = {}
            # bertS: rows shifted by one token (z roll); f32 exact for blend
            bertS = spool.tile([128, H], F32, tag="bertS")
            nc.sync.dma_start(bertS[:], bert[b, :, :])
            bertST = spool.tile([128, KC, 128], BF16, tag="bertST")
            nc.sync.dma_start(bertST[:], bertsT[b, :, :, :])
            st["bertST"] = bertST
            dept = dpool.tile([128, L, E], BF16, tag="dept")
            nc.sync.dma_start(dept[:, 0:64, :], dep[b, :, 0:64, :])
            nc.sync.dma_start(dept[:, 64:L, :], dep[b, :, 64:L, :])
            adjt = spool.tile([128, L], F32, tag="adjt")
            nc.sync.dma_start(adjt[:], adjf[b, :, :])
            st.update(bertS=bertS, dept=dept, adjt=adjt)

            # ---- zs^T = Wz @ bertS^T + bz ----
            p_z = p_big.tile([128, H], F32, tag="p_big")
            for hc in range(KC):
                ns = slice(hc * 128, (hc + 1) * 128)
                for kc in range(KC):
                    nc.tensor.matmul(p_z[:, ns], wzT[:, kc, ns],
                                     bertST[:, kc, :],
                                     start=(kc == 0), stop=False)
                nc.tensor.matmul(p_z[:, ns], bzr[0:1, ns], ones_b[:],
                                 start=False, stop=True)
            zsT = spool.tile([128, KC, 128], BF16, tag="zsT")
            nc.scalar.copy(zsT[:], p_z[:])
            if b == 0:
                dump("d_zsT", zsT[:])

            # ---- s_i col, (s_j + ba) row-bcast score base ----
            p_s3 = p_tr.tile([128, 512], F32, tag="p_tr")
            for kc in range(KC):
                nc.tensor.matmul(p_s3[0:1, 0:128], w2T[:, kc, 0:1],
                                 zsT[:, kc, :],
                                 start=(kc == 0), stop=(kc == KC - 1))
            for kc in range(KC):
                nc.tensor.matmul(p_s3[0:1, 128:256], w2T[:, kc, 1:2],
                                 zsT[:, kc, :],
                                 start=(kc == 0), stop=(kc == KC - 1))
            si_row = spool.tile([1, 128], F32, tag="si_row")
            nc.scalar.copy(si_row[:], p_s3[0:1, 0:128])
            sjb = spool.tile([1, 128], F32, tag="sjb")
            nc.vector.tensor_scalar(sjb[:], p_s3[0:1, 128:256], bar[0:1, 0:1],
                                    None, op0=OP.add)
            nc.tensor.matmul(p_s3[:, 384:385], si_row[:], ones_f[0:1, 0:1],
                             start=True, stop=True)
            nc.tensor.matmul(p_s3[:, 256:384], ones_f[:], sjb[:],
                             start=True, stop=True)
            if b == 0:
                dump("d_si", si_row[:])
                dump("d_sjb", sjb[:])

            # ---- s_e = reduce_e(dep * wa_e); dense mult split Pool/DVE ----
            tmp1 = tpool.tile([128, L, E], BF16, tag="ttmp")
            nc.gpsimd.tensor_tensor(
                tmp1[:, 0:JP1, :], dept[:, 0:JP1, :],
                wae_rep[:, 0:JP1, :], op=OP.mult)
            nc.vector.tensor_tensor(
                tmp1[:, JP1:L, :], dept[:, JP1:L, :],
                wae_rep[:, JP1:L, :], op=OP.mult)
            se = spool.tile([128, L], BF16, tag="se")
            nc.vector.tensor_reduce(se[:], tmp1[:], axis=AX.X, op=OP.add)
            if b == 0:
                dump("d_se", se[:])

            # ---- score = lrelu(se + si + sj + ba); masked; softmax ----
            sadd = spool.tile([128, L], F32, tag="sadd")
            nc.vector.scalar_tensor_tensor(
                sadd[:], se[:], p_s3[:, 384:385], p_s3[:, 256:384],
                op0=OP.add, op1=OP.add)
            score = spool.tile([128, L], F32, tag="score")
            nc.vector.scalar_tensor_tensor(
                score[:], sadd[:], 0.01, sadd[:], op0=OP.mult, op1=OP.max)
            masked = spool.tile([128, L], F32, tag="masked")
            nc.vector.scalar_tensor_tensor(
                masked[:], score[:], MASK_SHIFT, adjt[:],
                op0=OP.add, op1=OP.mult)
            if b == 0:
                dump("d_masked", masked[:])
            mxn = spool.tile([128, 1], F32, tag="mxn")
            nc.vector.tensor_reduce(mxn[:], masked[:], axis=AX.X, op=OP.max,
                                    negate=True)
            ex = spool.tile([128, L], F32, tag="ex")
            sumex = spool.tile([128, 1], F32, tag="sumex")
            nc.scalar.activation(ex[:], masked[:], AF.Exp, bias=mxn[:],
                                 scale=1.0, accum_out=sumex[:])
            rec = spool.tile([128, 1], F32, tag="rec")
            nc.vector.reciprocal(rec[:], sumex[:])
            attnb = spool.tile([128, L], BF16, tag="attnb")
            nc.vector.tensor_scalar(attnb[:], ex[:], rec[0:128, 0:1], None,
                                    op0=OP.mult)
            if b == 0:
                dump("d_attn", attnb[:])

            # attn^T transpose early (needs only attnb)
            p_ad = p_tr.tile([128, 256], BF16, tag="p_tr")
            nc.tensor.transpose(p_ad[:, 0:128], attnb[:], id_bf[:])
            attnT = spool.tile([128, 128], BF16, tag="attnT")
            nc.scalar.copy(attnT[:], p_ad[:, 0:128])
            st.update(zsT=zsT, attnb=attnb, p_ad=p_ad, attnT=attnT)
            return st

        def emit_back(b, st):
            zsT, dept, attnb = st["zsT"], st["dept"], st["attnb"]
            # ---- A2 = zs @ G0^T  (G0 = WhN @ WfZ host-folded) ----
            p_a = p_big.tile([128, H], F32, tag="p_big")
            for ns in (slice(0, 512), slice(512, H)):
                for kc in range(KC):
                    nc.tensor.matmul(p_a[:, ns], zsT[:, kc, :],
                                     g0T[:, kc, ns],
                                     start=(kc == 0), stop=(kc == KC - 1))
            a2b = spool.tile([128, H], BF16, tag="ab")
            nc.scalar.copy(a2b[:], p_a[:])

            # ---- temp partial: -bertS + zs@WhZ^T + attn@A2 (early) ----
            # temp(b) accumulates (temp - bertS) so the blend is one stt.
            p_t = p_big.tile([128, H], F32, tag="p_big")
            bertST_t = st["bertST"]
            for ns in (slice(0, 512), slice(512, H)):
                for kc in range(KC):
                    nc.tensor.matmul(p_t[:, ns], zsT[:, kc, :],
                                     whzT[:, kc, ns],
                                     start=(kc == 0), stop=False)
                nc.tensor.matmul(p_t[:, ns], st["attnT"][:], a2b[:, ns],
                                 start=False, stop=False)
            for hc in range(KC):
                ns = slice(hc * 128, (hc + 1) * 128)
                nc.tensor.matmul(p_t[:, ns], bertST_t[:, hc, :], id_negb[:],
                                 start=False, stop=False)

            # ---- D = reduce_j(attn * dep); Pool share densified via ACT ----
            arep = apool.tile([128, JPD, E], BF16, tag="arep")
            nc.scalar.copy(
                arep[:], attnb[:, 0:JPD].unsqueeze(2).broadcast_to(
                    [128, JPD, E]))
            tmp2 = tpool.tile([128, L, E], BF16, tag="ttmp")
            nc.gpsimd.tensor_tensor(
                tmp2[:, 0:JPD, :], dept[:, 0:JPD, :], arep[:], op=OP.mult)
            nc.vector.tensor_tensor(
                tmp2[:, JPD:L, :], dept[:, JPD:L, :],
                attnb[:, JPD:L].unsqueeze(2).broadcast_to([128, L - JPD, E]),
                op=OP.mult)
            # asymmetric pairwise tree: DVE folds j[0:96), Pool j[96:128)
            tmp3 = tpool.tile([128, L, E], BF16, tag="ttmp")
            t2, t3 = tmp2, tmp3
            nc.vector.tensor_tensor(t3[:, 0:48, :], t2[:, 0:48, :],
                                    t2[:, 48:96, :], op=OP.add)
            nc.gpsimd.tensor_tensor(t3[:, 96:112, :], t2[:, 96:112, :],
                                    t2[:, 112:128, :], op=OP.add)
            nc.vector.tensor_tensor(t2[:, 0:24, :], t3[:, 0:24, :],
                                    t3[:, 24:48, :], op=OP.add)
            nc.gpsimd.tensor_tensor(t2[:, 96:104, :], t3[:, 96:104, :],
                                    t3[:, 104:112, :], op=OP.add)
            nc.vector.tensor_tensor(t3[:, 0:12, :], t2[:, 0:12, :],
                                    t2[:, 12:24, :], op=OP.add)
            nc.gpsimd.tensor_tensor(t3[:, 96:100, :], t2[:, 96:100, :],
                                    t2[:, 100:104, :], op=OP.add)
            nc.vector.tensor_tensor(t2[:, 0:6, :], t3[:, 0:6, :],
                                    t3[:, 6:12, :], op=OP.add)
            nc.gpsimd.tensor_tensor(t2[:, 96:98, :], t3[:, 96:98, :],
                                    t3[:, 98:100, :], op=OP.add)
            nc.vector.tensor_tensor(t3[:, 0:3, :], t2[:, 0:3, :],
                                    t2[:, 3:6, :], op=OP.add)
            nc.gpsimd.tensor_tensor(t3[:, 96:97, :], t2[:, 96:97, :],
                                    t2[:, 97:98, :], op=OP.add)
            nc.vector.tensor_tensor(t2[:, 0:1, :], t3[:, 0:1, :],
                                    t3[:, 1:2, :], op=OP.add)
            nc.gpsimd.tensor_tensor(t2[:, 96:97, :], t3[:, 2:3, :],
                                    t3[:, 96:97, :], op=OP.add)
            dvb = spool.tile([128, E], BF16, tag="dvb")
            nc.vector.tensor_tensor(dvb[:], t2[:, 0, :], t2[:, 96, :],
                                    op=OP.add)
            if b == 0:
                dump("d_dvec", dvb[:])

            # D^T via PE transpose
            p_ad = st["p_ad"]
            nc.tensor.transpose(p_ad[0:E, 128:256], dvb[:], id_bf[:])
            dT = spool.tile([E, 128], BF16, tag="dT")
            nc.scalar.copy(dT[:], p_ad[0:E, 128:256])

            # ---- temp final: += D @ G1 (G1 = (WhN @ WfE)^T host-folded) ----
            for ns in (slice(0, 512), slice(512, H)):
                nc.tensor.matmul(p_t[:, ns], dT[:], g1[:, ns],
                                 start=False, stop=True)
            st["p_t"] = p_t

            # ---- upd mask column (host-folded span & any-neighbor) ----
            p_v = p_tr.tile([128, 512], F32, tag="p_tr")
            nc.tensor.matmul(p_v[:, 0:1], vrow4[0:1, b, :], ones_f[0:1, 0:1],
                             start=True, stop=True)

            # ---- blend: out = upd*(temp - bertS) + bertS; rolled store ----
            outt = opool.tile([128, H], F32, tag="outt")
            nc.vector.scalar_tensor_tensor(
                outt[:], p_t[:], p_v[:, 0:1], st["bertS"][:],
                op0=OP.mult, op1=OP.add)
            nc.sync.dma_start(out[b, 1:128, :], outt[0:127, :])
            nc.sync.dma_start(out[b, 0:1, :], outt[127:128, :])

        for b in range(PB):
            st = emit_front(b)
            emit_back(b, st)


def _get_nc():
    if "nc" not in _CACHED:
        _CACHED["nc"] = _build()
    return _CACHED["nc"]


def _chunkT(w):
    """W [rows, K] -> W^T chunk-major [128, K//128, rows] (lhsT layout)."""
    rows, k = w.shape
    return np.ascontiguousarray(
        w.T.reshape(k // 128, 128, rows).transpose(1, 0, 2))


def _prep_in_maps(bert_hidden_states, dep_type_adj, deprel_adj,
                  asp_start, asp_end, Wz, bz, wa, ba, Wf, Wh):
    bf = ml_dtypes.bfloat16
    bert = np.ascontiguousarray(np.asarray(bert_hidden_states, np.float32))
    dep = np.asarray(dep_type_adj, np.float32).astype(bf)
    adjf = np.ascontiguousarray(np.asarray(deprel_adj).astype(np.float32))
    # bertS^T chunk-major per batch: rows shifted by one (the z-roll)
    bs = np.ascontiguousarray(np.roll(bert, -1, axis=1))
    bertsT = np.ascontiguousarray(
        bs.transpose(0, 2, 1).reshape(B, KC, 128, L).transpose(0, 2, 1, 3)
    ).astype(bf)
    pos = np.arange(L, dtype=np.float32)
    s_ = np.asarray(asp_start).astype(np.float32)[:, None]
    e_ = np.asarray(asp_end).astype(np.float32)[:, None]
    vrow_full = (((pos[None, :] >= s_) & (pos[None, :] <= e_))
                 & (np.asarray(deprel_adj) > 0).any(-1)).astype(np.float32)

    Wz = np.asarray(Wz, np.float32)
    Wf = np.asarray(Wf, np.float32)
    Wh = np.asarray(Wh, np.float32)
    wa_f = np.asarray(wa, np.float32)
    wzT = _chunkT(Wz).astype(bf)
    g0T = _chunkT(Wh[:, :H] @ Wf[:, :H]).astype(bf)
    whzT = _chunkT(Wh[:, H:]).astype(bf)
    g1 = np.ascontiguousarray((Wh[:, :H] @ Wf[:, H:]).T).astype(bf)
    w2T = _chunkT(wa_f[:2 * H].reshape(2, H)).astype(bf)
    bzb = np.asarray(bz, np.float32)[None, :].astype(bf)
    waeb = wa_f[2 * H:][None, :].astype(bf)
    bab = np.asarray(ba, np.float32).reshape(1, 1)

    in_maps = []
    for c in range(NCORES):
        s = slice(c * PB, (c + 1) * PB)
        in_maps.append(dict(
            berts=bs[s], bertsT=np.ascontiguousarray(bertsT[s]),
            dep=dep[s], adjf=adjf[s],
            vrow=np.ascontiguousarray(vrow_full[s][None, :, :]),
            wzT=wzT, g0T=g0T, whzT=whzT, g1=g1, w2T=w2T,
            bzt=bzb, wae=waeb, bat=bab,
        ))
    return in_maps


def kernel(bert_hidden_states, dep_type_adj, deprel_adj, asp_start, asp_end,
           Wz, bz, wa, ba, Wf, Wh):
    from concourse.bass_utils import run_bass_kernel_spmd

    in_maps = _prep_in_maps(bert_hidden_states, dep_type_adj, deprel_adj,
                            asp_start, asp_end, Wz, bz, wa, ba, Wf, Wh)
    nc = _get_nc()
    res = run_bass_kernel_spmd(nc, in_maps, core_ids=list(range(NCORES)),
                               trace=bool(_CACHED.get("trace")),
                               tmpdir=_CACHED.get("trace_tmpdir"))
    _CACHED["last_results"] = res
    outs = [res.results[c]["out"] for c in range(NCORES)]
    return np.concatenate(outs, axis=0).astype(np.float32)

